# revision 1
# baseline (speedup 1.0000x reference)
"""CopyGenerator kernel for Trainium2, SPMD over 8 NeuronCores.

Problem (nn_CopyGenerator):
    logits = hidden @ W.T + b            # (N=4096, V=32000)
    prob   = softmax(logits, axis=1)
    p_copy = sigmoid(hidden @ Wc.T + bc) # (N, 1)
    out    = [prob * (1 - p_copy),  scatter(attn * p_copy)]   # (N, 32620)

Sharding: data-parallel over the batch axis. Core k handles batch elements
{4k..4k+3}; with local row r = j*128 + t (j = local batch idx, t = time).
W^T (with the bias appended as an extra row, applied via a K=1 matmul of
ones) is replicated to every core in fp16 and streamed twice — SBUF can
hold exp(logits) in fp16 for only two 128-row chunks at a time. The
row-sum partials come for free from the Exp activation's accum_out; the
per-row scale (1-p_copy)/rowsum is applied by DVE. Schedule: super-0
computes chunks 0,1 (PE-bound, DMA prefetches W and absorbs the
independent copy/scatter matmuls); super-1 computes chunks 2,3 while
chunk-0/1 stores are interleaved group-by-group with the W stream so the
DMA queue round-robins between loads and stores; the final chunk-2/3
drain is a pure DMA tail at full bandwidth.
"""

import numpy as np

import concourse.bass as bass
import concourse.mybir as mybir
import concourse.tile as tile
from concourse import bacc
from concourse.bass_utils import run_bass_kernel_spmd

# Problem constants (hardcoded per contract)
B, T, S, H, V, C = 32, 128, 400, 512, 32000, 620
N = B * T                  # 4096 total rows
NCORES = 8
BPC = B // NCORES          # batch elems per core = 4
ROWS = BPC * T             # rows per core = 512
P = 128                    # partitions
NT = 500                   # psum n-tile width (<=512 fp32 psum bank)
GW = 1000                  # W streaming group width (n-tiles per W DMA)
NTG = GW // NT             # n-tiles per group = 4
NGROUPS = V // GW          # 16 groups per pass
NTILES = V // NT           # 64 n-tiles per 128-row chunk
KK = H // P                # 4 contraction tiles
SUPERS = 2                 # W streamed once per super
CPS = BPC // SUPERS        # row-chunks per super = 2

FP16 = mybir.dt.float16
FP32 = mybir.dt.float32
AF = mybir.ActivationFunctionType


def build_kernel(bc_value: float):
    nc = bacc.Bacc("TRN2", target_bir_lowering=False)

    hT = nc.dram_tensor("hT", (H, ROWS), FP16, kind="ExternalInput")
    # W^T with the bias appended as row H (513, 32000)
    wTe = nc.dram_tensor("wTe", (H + 1, V), FP16, kind="ExternalInput")
    wc4 = nc.dram_tensor("wc4", (P, KK), FP16, kind="ExternalInput")
    aT = nc.dram_tensor("aT", (BPC, S, T), FP16, kind="ExternalInput")
    sm = nc.dram_tensor("sm", (BPC, S, C), FP16, kind="ExternalInput")
    out = nc.dram_tensor("out", (ROWS, V + C), FP32, kind="ExternalOutput")

    with tile.TileContext(nc) as tc:
        with (
            tc.tile_pool(name="const", bufs=1) as const,
            tc.tile_pool(name="wst", bufs=3) as wst,
            tc.tile_pool(name="expp", bufs=2 * NTILES + 4) as expp,
            tc.tile_pool(name="ost", bufs=4) as ost,
            tc.tile_pool(name="cstp", bufs=4) as cstp,
            tc.tile_pool(name="smallp", bufs=8) as smallp,
            tc.tile_pool(name="cpin", bufs=4) as cpin,
            tc.tile_pool(name="psmain", bufs=5, space="PSUM") as psmain,
            tc.tile_pool(name="pscopy", bufs=1, space="PSUM") as pscopy,
        ):
            # ---- resident constants ----
            hts = []
            for kk in range(KK):
                t = const.tile([P, ROWS], FP16, tag=f"ht{kk}")
                nc.sync.dma_start(t[:], hT[kk * P:(kk + 1) * P, :])
                hts.append(t)
            ones = const.tile([1, P], FP16, tag="ones")
            nc.vector.memset(ones[:], 1.0)
            bc_pos = const.tile([P, 1], FP32, tag="bcp")
            nc.vector.memset(bc_pos[:], bc_value)
            bc_neg = const.tile([P, 1], FP32, tag="bcn")
            nc.vector.memset(bc_neg[:], -bc_value)
            wc_sb = const.tile([P, KK], FP16, tag="wc")
            nc.sync.dma_start(wc_sb[:], wc4[:, :])
            accs = []
            for j in range(BPC):
                a = const.tile([P, NTILES], FP32, tag=f"acc{j}")
                accs.append(a)

            exp_tiles = [[None] * NTILES for _ in range(BPC)]
            s_tiles = [None] * BPC
            pc_tiles = [None] * BPC

            def emit_group(sup, g):
                """Load one W group and run matmul+exp for the super's chunks."""
                wts = []
                for kk in range(KK):
                    wt = wst.tile([P, GW], FP16, tag=f"w{kk}")
                    nc.sync.dma_start(
                        wt[:], wTe[kk * P:(kk + 1) * P, g * GW:(g + 1) * GW]
                    )
                    wts.append(wt)
                bt = wst.tile([1, GW], FP16, tag="brow")
                nc.sync.dma_start(bt[:], wTe[H:H + 1, g * GW:(g + 1) * GW])

                for cj in range(CPS):
                    j = sup * CPS + cj
                    js = slice(j * P, (j + 1) * P)
                    for n in range(NTG):
                        ps = psmain.tile([P, NT], FP32, tag="ps")
                        cs = slice(n * NT, (n + 1) * NT)
                        # bias broadcast via K=1 matmul of ones
                        nc.tensor.matmul(
                            ps[:], ones[:], bt[0:1, cs],
                            start=True, stop=False,
                        )
                        for kk in range(KK):
                            nc.tensor.matmul(
                                ps[:], hts[kk][:, js], wts[kk][:, cs],
                                start=False, stop=(kk == KK - 1),
                            )
                        e = expp.tile([P, NT], FP16, tag="e")
                        nidx = g * NTG + n
                        nc.scalar.activation(
                            e[:], ps[:], AF.Exp,
                            accum_out=accs[j][:, nidx:nidx + 1],
                        )
                        exp_tiles[j][nidx] = e

            ompc_tiles = [None] * BPC

            def emit_pc(j):
                """p_copy / (1-p_copy) for chunk j (independent of the
                vocab matmul, so traced up front)."""
                js = slice(j * P, (j + 1) * P)
                pps = pscopy.tile([P, 1], FP32, tag="pps")
                for kk in range(KK):
                    nc.tensor.matmul(
                        pps[:], hts[kk][:, js], wc_sb[:, kk:kk + 1],
                        start=(kk == 0), stop=(kk == KK - 1),
                    )
                pc = smallp.tile([P, 1], FP32, tag="pc")
                nc.scalar.activation(
                    pc[:], pps[:], AF.Sigmoid, bias=bc_pos[:], scale=1.0
                )
                ompc = smallp.tile([P, 1], FP32, tag="ompc")
                nc.scalar.activation(
                    ompc[:], pps[:], AF.Sigmoid, bias=bc_neg[:], scale=-1.0
                )
                pc_tiles[j] = pc
                ompc_tiles[j] = ompc

            def emit_head(j):
                """rowsum and the per-row scale for chunk j."""
                rs = smallp.tile([P, 1], FP32, tag="rs")
                nc.vector.reduce_sum(rs[:], accs[j][:], axis=mybir.AxisListType.X)
                rec = smallp.tile([P, 1], FP32, tag="rec")
                nc.vector.reciprocal(rec[:], rs[:])
                s = smallp.tile([P, 1], FP32, tag="s")
                nc.vector.tensor_mul(s[:], rec[:], ompc_tiles[j][:])
                s_tiles[j] = s

            def emit_store_group(j, go):
                """Scale one output group of chunk j and store it."""
                js = slice(j * P, (j + 1) * P)
                st = ost.tile([P, GW], FP32, tag="st")
                for q in range(NTG):
                    nidx = go * NTG + q
                    nc.vector.tensor_scalar_mul(
                        st[:, q * NT:(q + 1) * NT],
                        exp_tiles[j][nidx][:], s_tiles[j][:],
                    )
                nc.sync.dma_start(out[js, go * GW:(go + 1) * GW], st[:])

            def emit_copy(j):
                """copy/scatter part: p_copy * (attn_j @ src_map_j)."""
                js = slice(j * P, (j + 1) * P)
                cp1 = pscopy.tile([P, 512], FP32, tag="cp1")
                cp2 = pscopy.tile([P, C - 512], FP32, tag="cp2")
                nks = (S + P - 1) // P  # 4 (128,128,128,16)
                for kk in range(nks):
                    pk = min(P, S - kk * P)
                    at = cpin.tile([P, T], FP16, tag="at")
                    nc.sync.dma_start(at[:pk, :], aT[j, kk * P:kk * P + pk, :])
                    smt = cpin.tile([P, C], FP16, tag="smt")
                    nc.sync.dma_start(smt[:pk, :], sm[j, kk * P:kk * P + pk, :])
                    nc.tensor.matmul(
                        cp1[:], at[:pk, :], smt[:pk, 0:512],
                        start=(kk == 0), stop=(kk == nks - 1),
                    )
                    nc.tensor.matmul(
                        cp2[:], at[:pk, :], smt[:pk, 512:C],
                        start=(kk == 0), stop=(kk == nks - 1),
                    )
                cst = cstp.tile([P, C], FP32, tag="cst")
                nc.vector.tensor_scalar_mul(cst[:, 0:512], cp1[:], pc_tiles[j][:])
                nc.vector.tensor_scalar_mul(cst[:, 512:C], cp2[:], pc_tiles[j][:])
                nc.sync.dma_start(out[js, V:V + C], cst[:])

            # ---- p_copy first (cheap, needed by copies and heads) ----
            for j in range(BPC):
                emit_pc(j)

            # ---- super 0: compute chunks 0,1; sprinkle the independent
            # copy/scatter parts into the loop so their stores fill phase-1
            # DMA slack without a serial head ----
            for g in range(NGROUPS):
                emit_group(0, g)
                if g % 8 == 5:
                    emit_copy(g // 8)
            for j in range(CPS):
                emit_head(j)

            # ---- super 1: compute chunks 2,3 while draining chunks 0,1 ----
            # Interleave super-0 stores with super-1 W loads so the DMA queue
            # round-robins between them and PE never starves for W tiles.
            for g in range(NGROUPS):
                emit_group(1, g)
                for j in range(CPS):
                    emit_store_group(j, g)

            # ---- final epilogue: chunks 2,3 ----
            for cj in range(CPS):
                emit_head(CPS + cj)
            for go in range(NGROUPS):
                for cj in range(CPS):
                    emit_store_group(CPS + cj, go)

    nc.finalize()
    return nc


def _prep_inputs(hidden, attn, W, b, Wc, bc, src_map):
    """Host-side shard + layout prep. Returns per-core input maps and bc."""
    hidden, attn, W, b, Wc, bc, src_map = (
        np.asarray(x) for x in (hidden, attn, W, b, Wc, bc, src_map)
    )
    f16 = np.float16
    # W^T with bias appended as the last row, replicated
    wTe = np.empty((H + 1, V), dtype=f16)
    wTe[:H] = W.astype(f16).T
    wTe[H] = b.astype(f16)
    wc4 = np.ascontiguousarray(Wc[0].reshape(KK, P).T.astype(f16))  # (128, 4)

    hid = hidden.reshape(T, B, H)     # [t, b, h]
    att = attn.reshape(T, B, S)       # [t, b, s]

    in_maps = []
    for k in range(NCORES):
        bs = slice(k * BPC, (k + 1) * BPC)
        # local rows r = j*128 + t ; hT[h, r]
        hk = hid[:, bs, :].transpose(1, 0, 2).reshape(ROWS, H)   # [r, h]
        hT_k = np.ascontiguousarray(hk.T.astype(f16))            # (512, 512)
        aT_k = np.ascontiguousarray(
            att[:, bs, :].transpose(1, 2, 0).astype(f16))        # (4, S, T)
        sm_k = np.ascontiguousarray(
            src_map[:, bs, :].transpose(1, 0, 2).astype(f16))    # (4, S, C)
        in_maps.append({"hT": hT_k, "wTe": wTe, "wc4": wc4,
                        "aT": aT_k, "sm": sm_k})
    return in_maps, float(bc[0])


def _assemble(results):
    """Per-core (512, 32620) outputs -> full (4096, 32620)."""
    A = np.stack([r["out"] for r in results])       # (8, 512, V+C)
    A = A.reshape(NCORES, BPC, T, V + C)            # [k, j, t, :]
    A = A.transpose(2, 0, 1, 3).reshape(N, V + C)   # row = t*32 + (4k+j)
    return np.ascontiguousarray(A)


_CACHE = {}


def _run(inputs, **spmd_kwargs):
    in_maps, bc_value = _prep_inputs(**inputs)
    key = round(bc_value, 12)
    if key not in _CACHE:
        _CACHE[key] = build_kernel(bc_value)
    nc = _CACHE[key]
    res = run_bass_kernel_spmd(
        nc, in_maps, core_ids=list(range(NCORES)), **spmd_kwargs
    )
    return _assemble(res.results), res


def kernel(**inputs):
    out, _ = _run(inputs)
    return out



# revision 24
# speedup vs baseline: 2.5208x; 2.5208x over previous
"""CopyGenerator kernel for Trainium2, SPMD over 8 NeuronCores.

Problem (nn_CopyGenerator):
    logits = hidden @ W.T + b            # (N=4096, V=32000)
    prob   = softmax(logits, axis=1)
    p_copy = sigmoid(hidden @ Wc.T + bc) # (N, 1)
    out    = [prob * (1 - p_copy),  scatter(attn * p_copy)]   # (N, 32620)

Sharding: data-parallel over the batch axis. Core k handles batch elements
{4k..4k+3} (4 row-chunks of 128, local row r = j*128 + t).

Strategy (memory-regime; all numbers per core):
  * W is pre-scaled by 64 and stored as fp8e4 (16.4 MB instead of 32.8 fp16),
    hidden likewise; the vocab GEMM runs in DoubleRow fp8 (two 128-deep
    k-subtiles per matmul) which halves PE cycles again. The x64 scale is
    undone by the Exp activation's input scale.
  * The bias row is applied per 500-col tile via a K=1 matmul of fp16 ones
    against a streamed fp16 bias row.
  * exp(logits) is written as fp8 SBUF tiles; the Exp activation's accum_out
    yields the row-sum partials for free. V-part output is stored as fp8
    scaled by 4096 (the host divides it back out), the 620-col copy part as
    fp16 - cutting store traffic 4x vs fp32.
  * W streams twice (pass A: chunks 0,1; pass B: chunks 2,3). Chunks 0,1 are
    scaled (DVE+GPSIMD split) and stored while pass B computes; the tail
    scales chunks 2,3 on DVE+GPSIMD+ACT together.
  * p_copy uses a separate fp16 hidden copy for accuracy; sigmoid is computed
    as 1/(1+exp(-x)) on ACT+DVE to stay in the exp table set.
"""

import numpy as np
import ml_dtypes

import concourse.bass as bass
import concourse.mybir as mybir
import concourse.tile as tile
from concourse import bacc
from concourse.bass_utils import run_bass_kernel_spmd

# Problem constants (hardcoded per contract)
B, T, S, H, V, C = 32, 128, 400, 512, 32000, 620
N = B * T
NCORES = 8
BPC = B // NCORES          # batch elems (row-chunks) per core = 4
ROWS = BPC * T             # rows per core = 512
P = 128                    # partitions
KK = H // P                # 4 contraction subtiles of 128
NT = 500                   # psum n-tile width (bank holds 512 fp32)
GPC = 4                    # n-tiles per psum group
GW = NT * GPC              # 2000 cols per group
NG = V // GW               # 16 groups per chunk
SPAD = 512                 # source len padded to 4 subtiles
KS = SPAD // P             # 4
PASSES = 2
CPP = BPC // PASSES        # chunks per pass = 2

SCALE_W = 64.0             # W/b pre-scale (better fp8e4 range)
SCALE_OUT = 4096.0         # V-part output scale (host divides back)

FP8 = mybir.dt.float8e4
FP16 = mybir.dt.float16
FP32 = mybir.dt.float32
AF = mybir.ActivationFunctionType
DR = mybir.MatmulPerfMode.DoubleRow


def build_kernel(bc_value: float):
    nc = bacc.Bacc("TRN2", target_bir_lowering=False)

    h8 = nc.dram_tensor("h8", (P, KK, ROWS), FP8, kind="ExternalInput")
    h16 = nc.dram_tensor("h16", (P, KK, ROWS), FP16, kind="ExternalInput")
    w8 = nc.dram_tensor("w8", (P, KK, V), FP8, kind="ExternalInput")
    b16d = nc.dram_tensor("b16", (1, V), FP16, kind="ExternalInput")
    wc16d = nc.dram_tensor("wc16", (P, KK), FP16, kind="ExternalInput")
    a16 = nc.dram_tensor("a16", (BPC, P, KS, T), FP16, kind="ExternalInput")
    sm8 = nc.dram_tensor("sm8", (BPC, P, KS, C), FP8, kind="ExternalInput")
    outV = nc.dram_tensor("outV", (ROWS, V), FP8, kind="ExternalOutput")
    outC = nc.dram_tensor("outC", (ROWS, C), FP16, kind="ExternalOutput")

    with tile.TileContext(nc) as tc:
        with (
            tc.tile_pool(name="const", bufs=1) as const,
            tc.tile_pool(name="wst", bufs=4) as wst,
            tc.tile_pool(name="bst", bufs=4) as bst,
            tc.tile_pool(name="expp", bufs=PASSES * CPP * NG) as expp,
            tc.tile_pool(name="apool", bufs=4) as apool,
            tc.tile_pool(name="smpool", bufs=4) as smpool,
            tc.tile_pool(name="cpool", bufs=2) as cpool,
            tc.tile_pool(name="psp", bufs=2, space="PSUM") as psp,
        ):
            # ---- ramp-critical loads first: W group 0, then h8 ----
            w0 = wst.tile([P, KK, GW], FP8, tag="w")
            nc.sync.dma_start(w0[:], w8[:, :, 0:GW])
            b0 = bst.tile([1, GW], FP16, tag="b")
            nc.sync.dma_start(b0[:], b16d[0:1, 0:GW])
            h8t = const.tile([P, KK, ROWS], FP8, tag="h8t")
            nc.sync.dma_start(h8t[:], h8[:, :, :])
            ones16 = const.tile([1, P], FP16, tag="ones16")
            nc.vector.memset(ones16[:], 1.0)
            bcneg = const.tile([P, 1], FP32, tag="bcneg")
            nc.vector.memset(bcneg[:], -bc_value)
            accs = [
                const.tile([P, NG], FP32, tag=f"acc{j}", name=f"acc{j}")
                for j in range(BPC)
            ]

            exp_tiles = [[None] * NG for _ in range(BPC)]
            pcs = [None] * BPC
            ompc4 = [None] * BPC
            scales = [None] * BPC

            def emit_pc(pcps):
                """p_copy for all chunks into a group tile's slack columns
                (fp16 path, exp-based sigmoid to stay in the exp table set).
                Must be emitted BEFORE the host group's own matmuls so the
                slack isn't re-marked pending-zero afterwards."""
                for j in range(BPC):
                    js = slice(j * P, (j + 1) * P)
                    for kk in range(KK):
                        nc.tensor.matmul(
                            pcps[:, j:j + 1, 500:501],
                            h16t[:, kk:kk + 1, js],
                            wc16t[:, kk:kk + 1],
                            start=(kk == 0), stop=(kk == KK - 1),
                        )
                for j in range(BPC):
                    en = const.tile([P, 1], FP32, tag=f"en{j}", name=f"en{j}")
                    nc.scalar.activation(
                        en[:], pcps[:, j:j + 1, 500:501], AF.Exp,
                        bias=bcneg[:], scale=-1.0,
                    )
                    onep = const.tile([P, 1], FP32, tag=f"onep{j}", name=f"onep{j}")
                    nc.vector.tensor_scalar_add(onep[:], en[:], 1.0)
                    pc = const.tile([P, 1], FP32, tag=f"pc{j}", name=f"pc{j}")
                    nc.vector.reciprocal(pc[:], onep[:])          # sigmoid
                    om4 = const.tile([P, 1], FP32, tag=f"om4{j}", name=f"om4{j}")
                    nc.vector.tensor_mul(om4[:], en[:], pc[:])    # 1 - sigmoid
                    nc.vector.tensor_scalar_mul(om4[:], om4[:], SCALE_OUT)
                    pcs[j] = pc
                    ompc4[j] = om4

            def emit_copy(j, scale_eng):
                """copy/scatter part: p_copy * (attn_j @ src_map_j) -> fp16.
                scale_eng 0 = DVE, 2 = ACT (both can read psum)."""
                js = slice(j * P, (j + 1) * P)
                at, smt = copy_ins[j]
                cp = psp.tile([P, GPC, 512], FP32, tag="ps")
                for ks in range(KS):
                    nc.tensor.matmul(
                        cp[:, 0:1, 0:512], at[:, ks:ks + 1, :],
                        smt[:, ks:ks + 1, 0:512],
                        start=(ks == 0), stop=(ks == KS - 1),
                    )
                    nc.tensor.matmul(
                        cp[:, 1:2, 0:C - 512], at[:, ks:ks + 1, :],
                        smt[:, ks:ks + 1, 512:C],
                        start=(ks == 0), stop=(ks == KS - 1),
                    )
                cst = cpool.tile([P, C], FP16, tag="cst")
                if scale_eng == 0:
                    nc.vector.tensor_scalar_mul(cst[:, 0:512], cp[:, 0:1, 0:512], pcs[j][:])
                    nc.vector.tensor_scalar_mul(cst[:, 512:C], cp[:, 1:2, 0:C - 512], pcs[j][:])
                else:
                    nc.scalar.activation(cst[:, 0:512], cp[:, 0:1, 0:512], AF.Copy, scale=pcs[j][:])
                    nc.scalar.activation(cst[:, 512:C], cp[:, 1:2, 0:C - 512], AF.Copy, scale=pcs[j][:])
                nc.sync.dma_start(outC[js, :], cst[:])

            copy_ins = [None] * BPC

            def emit_copy_loads(j):
                at = apool.tile([P, KS, T], FP16, tag="at")
                nc.sync.dma_start(at[:], a16[j, :, :, :])
                smt = smpool.tile([P, KS, C], FP8, tag="smt")
                nc.sync.dma_start(smt[:], sm8[j, :, :, :])
                copy_ins[j] = (at, smt)

            def emit_wload(g):
                """Queue the DMA for one W group (+ bias slice)."""
                gs = slice(g * GW, (g + 1) * GW)
                wt = wst.tile([P, KK, GW], FP8, tag="w")
                nc.sync.dma_start(wt[:], w8[:, :, gs])
                bt = bst.tile([1, GW], FP16, tag="b")
                nc.sync.dma_start(bt[:], b16d[0:1, gs])
                return wt, bt

            def emit_group(pas, g, wt, bt, with_pc=False):
                """Matmul+exp for the pass's chunks against a loaded W group."""
                for cj in range(CPP):
                    j = pas * CPP + cj
                    js = slice(j * P, (j + 1) * P)
                    ps = psp.tile([P, GPC, 512], FP32, tag="ps")
                    if with_pc and cj == 0:
                        emit_pc(ps)
                    for q in range(GPC):
                        cs = slice(q * NT, (q + 1) * NT)
                        nc.tensor.matmul(
                            ps[:, q:q + 1, 0:NT], ones16[0:1, :], bt[0:1, cs],
                            start=True, stop=False,
                        )
                        for kh in range(KK // 2):
                            nc.tensor.matmul(
                                ps[:, q:q + 1, 0:NT],
                                h8t[:, 2 * kh:2 * kh + 2, js],
                                wt[:, 2 * kh:2 * kh + 2, cs],
                                start=False, stop=(kh == KK // 2 - 1),
                                perf_mode=DR,
                            )
                    et = expp.tile([P, GPC, NT], FP8, tag="e")
                    nc.scalar.activation(
                        et[:], ps[:, :, 0:NT], AF.Exp,
                        scale=1.0 / SCALE_W,
                    )
                    # Row-sum partial on DVE (x1.0 in place + accum), keeping
                    # the ACT engine free of the ~187ns accum-read per tile.
                    nc.vector.tensor_scalar(
                        et[:], et[:], 1.0, None,
                        mybir.AluOpType.mult, mybir.AluOpType.add,
                        accum_out=accs[j][:, g:g + 1],
                    )
                    exp_tiles[j][g] = et

            def emit_head(j):
                """Per-row output scale: 4096 * (1-p_copy) / rowsum."""
                rs = const.tile([P, 1], FP32, tag=f"rs{j}")
                nc.vector.reduce_sum(rs[:], accs[j][:], axis=mybir.AxisListType.X)
                rec = const.tile([P, 1], FP32, tag=f"rec{j}")
                nc.vector.reciprocal(rec[:], rs[:])
                s = const.tile([P, 1], FP32, tag=f"s{j}")
                nc.vector.tensor_mul(s[:], rec[:], ompc4[j][:])
                scales[j] = s

            def emit_scale(j, g, eng):
                """Scale one exp tile in place (0=DVE, 1=GPSIMD, 2=ACT)."""
                et = exp_tiles[j][g]
                if eng == 0:
                    nc.vector.tensor_scalar_mul(et[:], et[:], scales[j][:])
                elif eng == 1:
                    nc.gpsimd.tensor_scalar_mul(et[:], et[:], scales[j][:])
                else:
                    nc.scalar.activation(et[:], et[:], AF.Copy, scale=scales[j][:])

            def emit_store(j, g):
                """Store one scaled exp tile. Emitted a couple of ops after
                its scale: a dma_start whose source isn't ready blocks the
                SP sequencer FIFO and with it every later DMA issue."""
                js = slice(j * P, (j + 1) * P)
                nc.sync.dma_start(outV[js, g * GW:(g + 1) * GW], exp_tiles[j][g][:])

            # ---- main passes with W prefetch depth 2 across the pass
            # boundary (wst/bst pools are sized so loads run ~2 groups
            # ahead and pass B's first groups are in flight before pass A
            # drains). p_copy rides in group (A,2)'s psum slack; the
            # copy/scatter parts run in the tail where PE+psum are free. ----
            steps = [(pas, g) for pas in range(PASSES) for g in range(NG)]
            PF = 3
            loads = {0: (w0, b0)}
            for i in range(1, PF):
                loads[i] = emit_wload(steps[i][1])

            # Remaining resident constants (after the first W loads so the
            # ramp-critical transfers go out first).
            h16t = const.tile([P, KK, ROWS], FP16, tag="h16t")
            nc.sync.dma_start(h16t[:], h16[:, :, :])
            wc16t = const.tile([P, KK], FP16, tag="wc16t")
            nc.sync.dma_start(wc16t[:], wc16d[:, :])

            # Drain split for pass B: DVE/GPSIMD only (ACT is exp-bound).
            # DVE also accrues ~2.2us of accum work per group, modeled by
            # bumping its busy counter each step so the greedy interleaves
            # instead of front-loading GPSIMD.
            drain = [(cj, g) for g in range(NG) for cj in range(CPP)]
            dr_engs = []
            busy = [0.0, 0.0]
            cost2 = [1102.0, 2968.0]
            for s in range(NG):
                busy[0] += 2204.0
                for _ in range(CPP):
                    eng = min(range(2), key=lambda e: busy[e] + cost2[e])
                    dr_engs.append(eng)
                    busy[eng] += cost2[eng]
            di = 0
            store_q = []          # (j, g) scaled but not yet stored
            STORE_LAG = 2

            for i, (pas, g) in enumerate(steps):
                wt, bt = loads.pop(i)
                if i + PF < len(steps):
                    loads[i + PF] = emit_wload(steps[i + PF][1])
                emit_group(pas, g, wt, bt, with_pc=(pas == 0 and g == 2))
                if pas == 0 and g in (5, 7, 9, 11):
                    emit_copy_loads((g - 5) // 2)
                if pas == 0 and g == NG - 1:
                    for j in range(CPP):
                        emit_head(j)
                if pas == 1:
                    for _ in range(CPP):
                        j, gg = drain[di]
                        emit_scale(j, gg, dr_engs[di])
                        store_q.append((j, gg))
                        di += 1
                    while len(store_q) > STORE_LAG:
                        emit_store(*store_q.pop(0))
            for j in range(CPP):
                emit_head(CPP + j)

            # ---- tail: copy parts (PE+psum now free; scale DVE/ACT) and
            # chunks 2,3 scale+store across DVE/GPSIMD/ACT, greedy-balanced
            # (per-tile ns: DVE 1102, GPSIMD 2968, ACT 1852) ----
            for j in range(BPC):
                emit_copy(j, scale_eng=0 if j < 2 else 2)
            tail_engs = []
            busy3 = [3.1e3, 0.0, 3.4e3]   # seeded with the copy-scale work
            cost3 = [1102.0, 2968.0, 1852.0]
            for _ in range(2 * NG):
                eng = min(range(3), key=lambda e: busy3[e] + cost3[e])
                tail_engs.append(eng)
                busy3[eng] += cost3[eng]
            ti = 0
            for g in range(NG):
                for cj in range(CPP):
                    emit_scale(CPP + cj, g, tail_engs[ti])
                    store_q.append((CPP + cj, g))
                    ti += 1
                    while len(store_q) > STORE_LAG:
                        emit_store(*store_q.pop(0))
            while store_q:
                emit_store(*store_q.pop(0))

    nc.finalize()
    return nc


def _prep_inputs(hidden, attn, W, b, Wc, bc, src_map):
    """Host-side shard + layout prep. Returns per-core input maps and bc."""
    hidden, attn, W, b, Wc, bc, src_map = (
        np.asarray(x) for x in (hidden, attn, W, b, Wc, bc, src_map)
    )
    E4 = ml_dtypes.float8_e4m3
    f16 = np.float16

    # W^T * 64 in [p, kk, v] layout, fp8e4, replicated to all cores
    wT = (W.T.astype(np.float32) * SCALE_W).reshape(KK, P, V)
    w8 = np.ascontiguousarray(np.clip(wT, -240, 240).transpose(1, 0, 2)).astype(E4)
    b16 = (b.astype(np.float32) * SCALE_W).astype(f16).reshape(1, V)
    wc16 = np.ascontiguousarray(Wc[0].reshape(KK, P).T).astype(f16)  # (P, KK)

    hid = hidden.reshape(T, B, H)     # [t, b, h]
    att = attn.reshape(T, B, S)       # [t, b, s]

    in_maps = []
    for k in range(NCORES):
        bs = slice(k * BPC, (k + 1) * BPC)
        # local rows r = j*128 + t (j = local batch idx, t = time)
        hk = hid[:, bs, :].transpose(1, 0, 2).reshape(ROWS, H)   # [r, h]
        hT = hk.T.reshape(KK, P, ROWS).transpose(1, 0, 2)        # [p, kk, r]
        h8_k = np.ascontiguousarray(np.clip(hT, -240, 240)).astype(E4)
        h16_k = np.ascontiguousarray(hT).astype(f16)

        aT = att[:, bs, :].transpose(1, 2, 0)                    # (BPC, S, T)
        aP = np.zeros((BPC, SPAD, T), np.float32)
        aP[:, :S] = aT
        a16_k = np.ascontiguousarray(
            aP.reshape(BPC, KS, P, T).transpose(0, 2, 1, 3)).astype(f16)

        sK = src_map[:, bs, :].transpose(1, 0, 2)                # (BPC, S, C)
        sP = np.zeros((BPC, SPAD, C), np.float32)
        sP[:, :S] = sK
        sm8_k = np.ascontiguousarray(
            sP.reshape(BPC, KS, P, C).transpose(0, 2, 1, 3)).astype(E4)

        in_maps.append({"h8": h8_k, "h16": h16_k, "w8": w8, "b16": b16,
                        "wc16": wc16, "a16": a16_k, "sm8": sm8_k})
    return in_maps, float(bc[0])


def _assemble(results):
    """Per-core fp8 V-part (descale by 4096) + fp16 copy part -> (4096, 32620)."""
    Vp = np.stack([np.asarray(r["outV"]) for r in results])      # (8, 512, V)
    Cp = np.stack([np.asarray(r["outC"]) for r in results])      # (8, 512, C)
    out = np.empty((NCORES, ROWS, V + C), np.float32)
    out[:, :, :V] = Vp.astype(np.float32) * (1.0 / SCALE_OUT)
    out[:, :, V:] = Cp.astype(np.float32)
    out = out.reshape(NCORES, BPC, T, V + C)
    out = out.transpose(2, 0, 1, 3).reshape(N, V + C)            # row = t*32 + (4k+j)
    return np.ascontiguousarray(out)


_CACHE = {}


def _run(inputs, **spmd_kwargs):
    in_maps, bc_value = _prep_inputs(**inputs)
    key = round(bc_value, 12)
    if key not in _CACHE:
        _CACHE[key] = build_kernel(bc_value)
    nc = _CACHE[key]
    res = run_bass_kernel_spmd(
        nc, in_maps, core_ids=list(range(NCORES)), **spmd_kwargs
    )
    return _assemble(res.results), res


def kernel(**inputs):
    out, _ = _run(inputs)
    return out


# revision 53
# speedup vs baseline: 2.6002x; 1.0315x over previous
"""CopyGenerator kernel for Trainium2, SPMD over 8 NeuronCores.

Problem (nn_CopyGenerator):
    logits = hidden @ W.T + b            # (N=4096, V=32000)
    prob   = softmax(logits, axis=1)
    p_copy = sigmoid(hidden @ Wc.T + bc) # (N, 1)
    out    = [prob * (1 - p_copy),  scatter(attn * p_copy)]   # (N, 32620)

Sharding: data-parallel over the batch axis. Core k handles batch elements
{4k..4k+3} (4 row-chunks of 128, local row r = j*128 + t).

Strategy (memory-regime; all numbers per core):
  * W is pre-scaled by 64 and stored as fp8e4 (16.4 MB instead of 32.8 fp16),
    hidden likewise; the vocab GEMM runs in DoubleRow fp8 (two 128-deep
    k-subtiles per matmul) which halves PE cycles again. The x64 scale is
    undone by the Exp activation's input scale.
  * The bias row (x64, fp8) is applied per 500-col tile via a K=1 matmul of
    fp16 ones against the streamed bias row.
  * exp(logits) goes to fp8 SBUF tiles; row-sum partials come from a DVE
    in-place x1.0 with accum_out (keeping ACT free of accum reads). V-part
    output is stored as fp8 scaled by 4096 (the host divides it back out),
    the 620-col copy part as fp16 - 4x less store traffic than fp32.
  * W streams twice (pass A: chunks 0,1; pass B: chunks 2,3). Chunks 0,1 are
    scaled (DVE+GPSIMD) and stored while pass B computes; in pass B chunk 3
    runs 2 groups behind chunk 2 (W tiles live one rotation longer), so
    chunk 2's row-sum closes early and its stores overlap chunk 3's last
    exps; the tail scales the rest on DVE+GPSIMD+ACT. Stores are emitted a
    few ops behind their scales: a dma_start with an unready source blocks
    the SP issue FIFO and every later DMA behind it.
  * W groups 0-1 load in half-width transfers and drain in two 2-bank ACT
    halves (spare accum columns NG+g), pulling the exp stream start to
    ~5us; after the W stream ends the kernel is ~94% store-bound.
  * p_copy uses a separate fp16 hidden copy for accuracy; sigmoid is computed
    as 1/(1+exp(-x)) on ACT+DVE to stay in the exp table set.
"""

import numpy as np
import ml_dtypes

import concourse.bass as bass
import concourse.mybir as mybir
import concourse.tile as tile
from concourse import bacc
from concourse.bass_utils import run_bass_kernel_spmd

# Problem constants (hardcoded per contract)
B, T, S, H, V, C = 32, 128, 400, 512, 32000, 620
N = B * T
NCORES = 8
BPC = B // NCORES          # batch elems (row-chunks) per core = 4
ROWS = BPC * T             # rows per core = 512
P = 128                    # partitions
KK = H // P                # 4 contraction subtiles of 128
NT = 500                   # psum n-tile width (bank holds 512 fp32)
GPC = 4                    # n-tiles per psum group
GW = NT * GPC              # 2000 cols per group
NG = V // GW               # 16 groups per chunk
SPAD = 512                 # source len padded to 4 subtiles
KS = SPAD // P             # 4
PASSES = 2
CPP = BPC // PASSES        # chunks per pass = 2

SCALE_W = 64.0             # W/b pre-scale (better fp8e4 range)
SCALE_OUT = 4096.0         # V-part output scale (host divides back)

FP8 = mybir.dt.float8e4
FP16 = mybir.dt.float16
FP32 = mybir.dt.float32
AF = mybir.ActivationFunctionType
DR = mybir.MatmulPerfMode.DoubleRow


def build_kernel(bc_value: float):
    nc = bacc.Bacc("TRN2", target_bir_lowering=False)

    h8 = nc.dram_tensor("h8", (P, KK, ROWS), FP8, kind="ExternalInput")
    h16 = nc.dram_tensor("h16", (P, KK, ROWS), FP16, kind="ExternalInput")
    w8 = nc.dram_tensor("w8", (P, KK, V), FP8, kind="ExternalInput")
    b8d = nc.dram_tensor("b8", (1, V), FP8, kind="ExternalInput")
    wc16d = nc.dram_tensor("wc16", (P, KK), FP16, kind="ExternalInput")
    a16 = nc.dram_tensor("a16", (BPC, P, KS, T), FP16, kind="ExternalInput")
    sm8 = nc.dram_tensor("sm8", (BPC, P, KS, C), FP8, kind="ExternalInput")
    outV = nc.dram_tensor("outV", (ROWS, V), FP8, kind="ExternalOutput")
    outC = nc.dram_tensor("outC", (ROWS, C), FP16, kind="ExternalOutput")

    with tile.TileContext(nc) as tc:
        with (
            tc.tile_pool(name="const", bufs=1) as const,
            tc.tile_pool(name="wst", bufs=5) as wst,
            tc.tile_pool(name="bst", bufs=5) as bst,
            tc.tile_pool(name="expp", bufs=PASSES * CPP * NG) as expp,
            tc.tile_pool(name="apool", bufs=4) as apool,
            tc.tile_pool(name="smpool", bufs=4) as smpool,
            tc.tile_pool(name="cpool", bufs=2) as cpool,
            tc.tile_pool(name="psp", bufs=2, space="PSUM") as psp,
        ):
            # ---- ramp-critical loads first: W group 0 in two half-width
            # transfers (the first exp only needs the first half), then h8.
            # The tiny bias row goes after w0's first half: every DMA ahead
            # of w0a costs serial HWDGE+transfer time on the ramp. ----
            w0 = wst.tile([P, KK, GW], FP8, tag="w")
            nc.sync.dma_start(w0[:, :, 0:GW // 2], w8[:, :, 0:GW // 2])
            b0 = bst.tile([1, GW], FP8, tag="b")
            nc.sync.dma_start(b0[:], b8d[0:1, 0:GW])
            h8t = const.tile([P, KK, ROWS], FP8, tag="h8t")
            nc.sync.dma_start(h8t[:], h8[:, :, :])
            nc.sync.dma_start(w0[:, :, GW // 2:GW], w8[:, :, GW // 2:GW])
            ones16 = const.tile([1, P], FP16, tag="ones16")
            nc.vector.memset(ones16[:], 1.0)
            bcneg = const.tile([P, 1], FP32, tag="bcneg")
            nc.vector.memset(bcneg[:], -bc_value)
            accs = [
                const.tile([P, NG + 2], FP32, tag=f"acc{j}", name=f"acc{j}")
                for j in range(BPC)
            ]
            # Spare columns NG+g hold the half-B partials of split groups
            # g in (0,1); zero them everywhere first (accum_out overwrites).
            for j in range(BPC):
                nc.vector.memset(accs[j][:, NG:NG + 2], 0.0)

            exp_tiles = [[None] * NG for _ in range(BPC)]
            pcs = [None] * BPC
            ompc4 = [None] * BPC
            scales = [None] * BPC

            def emit_pc(pcps):
                """p_copy for all chunks into a group tile's slack columns
                (fp16 path, exp-based sigmoid to stay in the exp table set).
                Must be emitted BEFORE the host group's own matmuls so the
                slack isn't re-marked pending-zero afterwards."""
                for j in range(BPC):
                    js = slice(j * P, (j + 1) * P)
                    for kk in range(KK):
                        nc.tensor.matmul(
                            pcps[:, j:j + 1, 500:501],
                            h16t[:, kk:kk + 1, js],
                            wc16t[:, kk:kk + 1],
                            start=(kk == 0), stop=(kk == KK - 1),
                        )
                for j in range(BPC):
                    en = const.tile([P, 1], FP32, tag=f"en{j}", name=f"en{j}")
                    nc.scalar.activation(
                        en[:], pcps[:, j:j + 1, 500:501], AF.Exp,
                        bias=bcneg[:], scale=-1.0,
                    )
                    onep = const.tile([P, 1], FP32, tag=f"onep{j}", name=f"onep{j}")
                    nc.vector.tensor_scalar_add(onep[:], en[:], 1.0)
                    pc = const.tile([P, 1], FP32, tag=f"pc{j}", name=f"pc{j}")
                    nc.vector.reciprocal(pc[:], onep[:])          # sigmoid
                    om4 = const.tile([P, 1], FP32, tag=f"om4{j}", name=f"om4{j}")
                    nc.vector.tensor_mul(om4[:], en[:], pc[:])    # 1 - sigmoid
                    nc.vector.tensor_scalar_mul(om4[:], om4[:], SCALE_OUT)
                    pcs[j] = pc
                    ompc4[j] = om4

            def emit_copy(j, scale_eng):
                """copy/scatter part: p_copy * (attn_j @ src_map_j) -> fp16.
                scale_eng 0 = DVE, 2 = ACT (both can read psum)."""
                js = slice(j * P, (j + 1) * P)
                at, smt = copy_ins[j]
                cp = psp.tile([P, GPC, 512], FP32, tag="ps")
                for ks in range(KS):
                    nc.tensor.matmul(
                        cp[:, 0:1, 0:512], at[:, ks:ks + 1, :],
                        smt[:, ks:ks + 1, 0:512],
                        start=(ks == 0), stop=(ks == KS - 1),
                    )
                    nc.tensor.matmul(
                        cp[:, 1:2, 0:C - 512], at[:, ks:ks + 1, :],
                        smt[:, ks:ks + 1, 512:C],
                        start=(ks == 0), stop=(ks == KS - 1),
                    )
                cst = cpool.tile([P, C], FP16, tag="cst")
                if scale_eng == 0:
                    nc.vector.tensor_scalar_mul(cst[:, 0:512], cp[:, 0:1, 0:512], pcs[j][:])
                    nc.vector.tensor_scalar_mul(cst[:, 512:C], cp[:, 1:2, 0:C - 512], pcs[j][:])
                else:
                    nc.scalar.activation(cst[:, 0:512], cp[:, 0:1, 0:512], AF.Copy, scale=pcs[j][:])
                    nc.scalar.activation(cst[:, 512:C], cp[:, 1:2, 0:C - 512], AF.Copy, scale=pcs[j][:])
                nc.sync.dma_start(outC[js, :], cst[:])
                return js, cst

            copy_ins = [None] * BPC

            def emit_copy_loads(j):
                at = apool.tile([P, KS, T], FP16, tag="at")
                nc.sync.dma_start(at[:], a16[j, :, :, :])
                smt = smpool.tile([P, KS, C], FP8, tag="smt")
                nc.sync.dma_start(smt[:], sm8[j, :, :, :])
                copy_ins[j] = (at, smt)

            RES_GS = ()
            resident = {}

            def emit_wload(pas_g):
                """Queue the DMA for one W group (+ bias slice). Groups in
                RES_GS are loaded once in pass A into persistent tiles and
                reused by pass B, trimming pass-B DMA and removing the
                W-arrival wait at the end of the kernel."""
                pas, g = pas_g
                if pas == 1 and g in RES_GS:
                    return resident[g]
                gs = slice(g * GW, (g + 1) * GW)
                if pas == 0 and g in RES_GS:
                    wt = const.tile([P, KK, GW], FP8, tag=f"wres{g}", name=f"wres{g}")
                    bt = const.tile([1, GW], FP8, tag=f"bres{g}", name=f"bres{g}")
                    resident[g] = (wt, bt)
                else:
                    wt = wst.tile([P, KK, GW], FP8, tag="w")
                    bt = bst.tile([1, GW], FP8, tag="b")
                if pas == 0 and g == 1:
                    nc.sync.dma_start(wt[:, :, 0:GW // 2], w8[:, :, gs][:, :, 0:GW // 2])
                    nc.sync.dma_start(bt[:], b8d[0:1, gs])
                    nc.sync.dma_start(wt[:, :, GW // 2:GW], w8[:, :, gs][:, :, GW // 2:GW])
                else:
                    nc.sync.dma_start(wt[:], w8[:, :, gs])
                    nc.sync.dma_start(bt[:], b8d[0:1, gs])
                return wt, bt

            def emit_chunk(j, g, wt, bt, with_pc=False, split=False):
                """Matmul+exp for one chunk against a loaded W group.
                split: drain in two 2-bank halves so the very first exp
                starts as soon as the first half-W transfer lands."""
                js = slice(j * P, (j + 1) * P)
                ps = psp.tile([P, GPC, 512], FP32, tag="ps")
                if with_pc:
                    emit_pc(ps)
                halves = 2 if split else 1
                qph = GPC // halves
                et = expp.tile([P, GPC, NT], FP8, tag="e")
                for h in range(halves):
                    for q in range(h * qph, (h + 1) * qph):
                        cs = slice(q * NT, (q + 1) * NT)
                        nc.tensor.matmul(
                            ps[:, q:q + 1, 0:NT], ones16[0:1, :], bt[0:1, cs],
                            start=True, stop=False,
                        )
                        for kh in range(KK // 2):
                            nc.tensor.matmul(
                                ps[:, q:q + 1, 0:NT],
                                h8t[:, 2 * kh:2 * kh + 2, js],
                                wt[:, 2 * kh:2 * kh + 2, cs],
                                start=False, stop=(kh == KK // 2 - 1),
                                perf_mode=DR,
                            )
                    hq = slice(h * qph, (h + 1) * qph)
                    nc.scalar.activation(
                        et[:, hq, :], ps[:, hq, 0:NT], AF.Exp,
                        scale=1.0 / SCALE_W,
                    )
                    # Row-sum partial on DVE (x1.0 in place + accum),
                    # keeping ACT free of the ~187ns accum-read per tile.
                    # Split halves use the spare accum column NG.
                    acol = g if h == 0 else NG + g
                    nc.vector.tensor_scalar(
                        et[:, hq, :], et[:, hq, :], 1.0, None,
                        mybir.AluOpType.mult, mybir.AluOpType.add,
                        accum_out=accs[j][:, acol:acol + 1],
                    )
                exp_tiles[j][g] = et

            def emit_head(j):
                """Per-row output scale: 4096 * (1-p_copy) / rowsum."""
                rs = const.tile([P, 1], FP32, tag=f"rs{j}")
                nc.vector.reduce_sum(rs[:], accs[j][:], axis=mybir.AxisListType.X)
                rec = const.tile([P, 1], FP32, tag=f"rec{j}")
                nc.vector.reciprocal(rec[:], rs[:])
                s = const.tile([P, 1], FP32, tag=f"s{j}")
                nc.vector.tensor_mul(s[:], rec[:], ompc4[j][:])
                scales[j] = s

            def emit_scale(j, g, eng):
                """Scale one exp tile in place (0=DVE, 1=GPSIMD, 2=ACT)."""
                et = exp_tiles[j][g]
                if eng == 0:
                    nc.vector.tensor_scalar_mul(et[:], et[:], scales[j][:])
                elif eng == 1:
                    nc.gpsimd.tensor_scalar_mul(et[:], et[:], scales[j][:])
                else:
                    nc.scalar.activation(et[:], et[:], AF.Copy, scale=scales[j][:])

            def emit_store(j, g):
                """Store one scaled exp tile. Emitted a couple of ops after
                its scale: a dma_start whose source isn't ready blocks the
                SP sequencer FIFO and with it every later DMA issue."""
                js = slice(j * P, (j + 1) * P)
                nc.sync.dma_start(outV[js, g * GW:(g + 1) * GW], exp_tiles[j][g][:])

            # ---- main passes with W prefetch depth 2 across the pass
            # boundary (wst/bst pools are sized so loads run ~2 groups
            # ahead and pass B's first groups are in flight before pass A
            # drains). p_copy rides in group (A,2)'s psum slack; the
            # copy/scatter parts run in the tail where PE+psum are free. ----
            steps = [(pas, g) for pas in range(PASSES) for g in range(NG)]
            PF = 3
            loads = {0: (w0, b0)}
            lstate = {"next": 1}

            def issue_loads(upto):
                while lstate["next"] < min(upto, len(steps)):
                    loads[lstate["next"]] = emit_wload(steps[lstate["next"]])
                    lstate["next"] += 1

            issue_loads(PF)

            # Remaining resident constants (after the first W loads so the
            # ramp-critical transfers go out first).
            h16t = const.tile([P, KK, ROWS], FP16, tag="h16t")
            nc.sync.dma_start(h16t[:], h16[:, :, :])
            wc16t = const.tile([P, KK], FP16, tag="wc16t")
            nc.sync.dma_start(wc16t[:], wc16d[:, :])

            # Drain split for pass B: DVE/GPSIMD only (ACT is exp-bound).
            # DVE also accrues ~2.2us of accum work per slot, modeled by
            # bumping its busy counter each step so the greedy interleaves
            # instead of front-loading GPSIMD.
            drain = [(cj, g) for g in range(NG) for cj in range(CPP)]
            dr_engs = []
            busy = [0.0, 0.0]
            cost2 = [1102.0, 2968.0]
            for s in range(NG):
                busy[0] += 2204.0
                for _ in range(CPP):
                    eng = min(range(2), key=lambda e: busy[e] + cost2[e])
                    dr_engs.append(eng)
                    busy[eng] += cost2[eng]
            di = 0
            store_q = []          # (j, g) scaled but not yet stored
            STORE_LAG = 6

            # ---- pass A: chunks 0,1 group-by-group ----
            for g in range(NG):
                wt, bt = loads.pop(g)
                issue_loads(g + 1 + PF)
                emit_chunk(0, g, wt, bt, with_pc=(g == 2), split=(g <= 1))
                emit_chunk(1, g, wt, bt, split=(g <= 1))
                if g in (5, 7, 9, 11):
                    emit_copy_loads((g - 5) // 2)
            for j in range(CPP):
                emit_head(j)

            # ---- pass B: chunk 3 staggered K_STAG groups behind chunk 2,
            # so chunk 2's row-sum completes early and its scale+stores
            # overlap chunk 3's last exps. W tiles stay alive one rotation
            # longer (wst/bst bufs cover PF-ahead + K_STAG-behind). ----
            K_STAG = 2
            for s in range(NG + K_STAG):
                issue_loads(NG + s + K_STAG + 1)
                if s < NG:
                    wt, bt = loads[NG + s]
                    emit_chunk(2, s, wt, bt)
                if s >= K_STAG:
                    gg = s - K_STAG
                    wt, bt = loads.pop(NG + gg)
                    emit_chunk(3, gg, wt, bt)
                if s < NG:
                    for _ in range(CPP):
                        j, gg2 = drain[di]
                        emit_scale(j, gg2, dr_engs[di])
                        store_q.append((j, gg2))
                        di += 1
                    while len(store_q) > STORE_LAG:
                        emit_store(*store_q.pop(0))
                if s == NG - 1:
                    emit_head(CPP)
            emit_head(CPP + 1)

            # ---- tail: copy parts (PE+psum now free; scale DVE/ACT) and
            # chunks 2,3 scale+store across DVE/GPSIMD/ACT, greedy-balanced
            # (per-tile ns: DVE 1102, GPSIMD 2968, ACT 1852) ----
            while store_q:
                emit_store(*store_q.pop(0))
            copy_sts = [emit_copy(j, scale_eng=0 if j < 2 else 2)
                        for j in range(BPC)]
            tail_engs = []
            busy3 = [3.1e3, 0.0, 3.4e3]   # seeded with the copy-scale work
            cost3 = [1102.0, 2968.0, 1852.0]
            for _ in range(2 * NG):
                eng = min(range(3), key=lambda e: busy3[e] + cost3[e])
                tail_engs.append(eng)
                busy3[eng] += cost3[eng]
            # No W loads remain, so a store blocking the SP queue can only
            # delay other stores - emit each store right after its scale;
            # the copy parts' outC stores go out once their data is ready.
            ti = 0
            for cj in range(CPP):
                for g in range(NG):
                    emit_scale(CPP + cj, g, tail_engs[ti])
                    emit_store(CPP + cj, g)
                    ti += 1


    nc.finalize()
    return nc


def _prep_inputs(hidden, attn, W, b, Wc, bc, src_map):
    """Host-side shard + layout prep. Returns per-core input maps and bc."""
    hidden, attn, W, b, Wc, bc, src_map = (
        np.asarray(x) for x in (hidden, attn, W, b, Wc, bc, src_map)
    )
    E4 = ml_dtypes.float8_e4m3
    f16 = np.float16

    # W^T * 64 in [p, kk, v] layout, fp8e4, replicated to all cores
    wT = (W.T.astype(np.float32) * SCALE_W).reshape(KK, P, V)
    w8 = np.ascontiguousarray(np.clip(wT, -240, 240).transpose(1, 0, 2)).astype(E4)
    b8 = np.clip(b.astype(np.float32) * SCALE_W, -240, 240).astype(E4).reshape(1, V)
    wc16 = np.ascontiguousarray(Wc[0].reshape(KK, P).T).astype(f16)  # (P, KK)

    hid = hidden.reshape(T, B, H)     # [t, b, h]
    att = attn.reshape(T, B, S)       # [t, b, s]

    in_maps = []
    for k in range(NCORES):
        bs = slice(k * BPC, (k + 1) * BPC)
        # local rows r = j*128 + t (j = local batch idx, t = time)
        hk = hid[:, bs, :].transpose(1, 0, 2).reshape(ROWS, H)   # [r, h]
        hT = hk.T.reshape(KK, P, ROWS).transpose(1, 0, 2)        # [p, kk, r]
        h8_k = np.ascontiguousarray(np.clip(hT, -240, 240)).astype(E4)
        h16_k = np.ascontiguousarray(hT).astype(f16)

        aT = att[:, bs, :].transpose(1, 2, 0)                    # (BPC, S, T)
        aP = np.zeros((BPC, SPAD, T), np.float32)
        aP[:, :S] = aT
        a16_k = np.ascontiguousarray(
            aP.reshape(BPC, KS, P, T).transpose(0, 2, 1, 3)).astype(f16)

        sK = src_map[:, bs, :].transpose(1, 0, 2)                # (BPC, S, C)
        sP = np.zeros((BPC, SPAD, C), np.float32)
        sP[:, :S] = sK
        sm8_k = np.ascontiguousarray(
            sP.reshape(BPC, KS, P, C).transpose(0, 2, 1, 3)).astype(E4)

        in_maps.append({"h8": h8_k, "h16": h16_k, "w8": w8, "b8": b8,
                        "wc16": wc16, "a16": a16_k, "sm8": sm8_k})
    return in_maps, float(bc[0])


def _assemble(results):
    """Per-core fp8 V-part (descale by 4096) + fp16 copy part -> (4096, 32620)."""
    Vp = np.stack([np.asarray(r["outV"]) for r in results])      # (8, 512, V)
    Cp = np.stack([np.asarray(r["outC"]) for r in results])      # (8, 512, C)
    out = np.empty((NCORES, ROWS, V + C), np.float32)
    out[:, :, :V] = Vp.astype(np.float32) * (1.0 / SCALE_OUT)
    out[:, :, V:] = Cp.astype(np.float32)
    out = out.reshape(NCORES, BPC, T, V + C)
    out = out.transpose(2, 0, 1, 3).reshape(N, V + C)            # row = t*32 + (4k+j)
    return np.ascontiguousarray(out)


_CACHE = {}


def _run(inputs, **spmd_kwargs):
    in_maps, bc_value = _prep_inputs(**inputs)
    key = round(bc_value, 12)
    if key not in _CACHE:
        _CACHE[key] = build_kernel(bc_value)
    nc = _CACHE[key]
    res = run_bass_kernel_spmd(
        nc, in_maps, core_ids=list(range(NCORES)), **spmd_kwargs
    )
    return _assemble(res.results), res


def kernel(**inputs):
    out, _ = _run(inputs)
    return out


# revision 54
# speedup vs baseline: 2.6355x; 1.0136x over previous
"""CopyGenerator kernel for Trainium2, SPMD over 8 NeuronCores.

Problem (nn_CopyGenerator):
    logits = hidden @ W.T + b            # (N=4096, V=32000)
    prob   = softmax(logits, axis=1)
    p_copy = sigmoid(hidden @ Wc.T + bc) # (N, 1)
    out    = [prob * (1 - p_copy),  scatter(attn * p_copy)]   # (N, 32620)

Sharding: data-parallel over the batch axis. Core k handles batch elements
{4k..4k+3} (4 row-chunks of 128, local row r = j*128 + t).

Strategy (memory-regime; all numbers per core):
  * W is pre-scaled by 64 and stored as fp8e4 (16.4 MB instead of 32.8 fp16),
    hidden likewise; the vocab GEMM runs in DoubleRow fp8 (two 128-deep
    k-subtiles per matmul) which halves PE cycles again. The x64 scale is
    undone by the Exp activation's input scale.
  * The bias row (x64, fp8) is applied per 500-col tile via a K=1 matmul of
    fp16 ones against the streamed bias row.
  * exp(logits) goes to fp8 SBUF tiles; row-sum partials come from a DVE
    in-place x1.0 with accum_out (keeping ACT free of accum reads). V-part
    output is stored as fp8 scaled by 4096 (the host divides it back out),
    the 620-col copy part as fp16 - 4x less store traffic than fp32.
  * W streams twice (pass A: chunks 0,1; pass B: chunks 2,3). Chunks 0,1 are
    scaled (DVE+GPSIMD) and stored while pass B computes; in pass B chunk 3
    runs 2 groups behind chunk 2 (W tiles live one rotation longer), so
    chunk 2's row-sum closes early and its stores overlap chunk 3's last
    exps; the tail scales the rest on DVE+GPSIMD+ACT. Stores are emitted a
    few ops behind their scales: a dma_start with an unready source blocks
    the SP issue FIFO and every later DMA behind it.
  * W groups 0-1 load in half-width transfers and drain in two 2-bank ACT
    halves (spare accum columns NG+g), pulling the exp stream start to
    ~5us; after the W stream ends the kernel is ~94% store-bound.
  * p_copy uses a separate fp16 hidden copy for accuracy; sigmoid is computed
    as 1/(1+exp(-x)) on ACT+DVE to stay in the exp table set.
"""

import numpy as np
import ml_dtypes

import concourse.bass as bass
import concourse.mybir as mybir
import concourse.tile as tile
from concourse import bacc
from concourse.bass_utils import run_bass_kernel_spmd

# Problem constants (hardcoded per contract)
B, T, S, H, V, C = 32, 128, 400, 512, 32000, 620
N = B * T
NCORES = 8
BPC = B // NCORES          # batch elems (row-chunks) per core = 4
ROWS = BPC * T             # rows per core = 512
P = 128                    # partitions
KK = H // P                # 4 contraction subtiles of 128
NT = 500                   # psum n-tile width (bank holds 512 fp32)
GPC = 4                    # n-tiles per psum group
GW = NT * GPC              # 2000 cols per group
NG = V // GW               # 16 groups per chunk
SPAD = 512                 # source len padded to 4 subtiles
KS = SPAD // P             # 4
PASSES = 2
CPP = BPC // PASSES        # chunks per pass = 2

SCALE_W = 64.0             # W/b pre-scale (better fp8e4 range)
SCALE_OUT = 4096.0         # V-part output scale (host divides back)

FP8 = mybir.dt.float8e4
FP16 = mybir.dt.float16
FP32 = mybir.dt.float32
AF = mybir.ActivationFunctionType
DR = mybir.MatmulPerfMode.DoubleRow


def build_kernel(bc_value: float):
    nc = bacc.Bacc("TRN2", target_bir_lowering=False)

    h8 = nc.dram_tensor("h8", (P, KK, ROWS), FP8, kind="ExternalInput")
    h16 = nc.dram_tensor("h16", (P, KK, ROWS), FP16, kind="ExternalInput")
    w8 = nc.dram_tensor("w8", (P, KK, V), FP8, kind="ExternalInput")
    b8d = nc.dram_tensor("b8", (1, V), FP8, kind="ExternalInput")
    wc16d = nc.dram_tensor("wc16", (P, KK), FP16, kind="ExternalInput")
    a16 = nc.dram_tensor("a16", (BPC, P, KS, T), FP16, kind="ExternalInput")
    sm8 = nc.dram_tensor("sm8", (BPC, P, KS, C), FP8, kind="ExternalInput")
    outV = nc.dram_tensor("outV", (ROWS, V), FP8, kind="ExternalOutput")
    outC = nc.dram_tensor("outC", (ROWS, C), FP16, kind="ExternalOutput")

    with tile.TileContext(nc) as tc:
        with (
            tc.tile_pool(name="const", bufs=1) as const,
            tc.tile_pool(name="wst", bufs=5) as wst,
            tc.tile_pool(name="bst", bufs=5) as bst,
            tc.tile_pool(name="expp", bufs=PASSES * CPP * NG) as expp,
            tc.tile_pool(name="apool", bufs=4) as apool,
            tc.tile_pool(name="smpool", bufs=4) as smpool,
            tc.tile_pool(name="cpool", bufs=2) as cpool,
            tc.tile_pool(name="psp", bufs=2, space="PSUM") as psp,
        ):
            # ---- ramp-critical loads first: W group 0 in two half-width
            # transfers (the first exp only needs the first half), then h8.
            # The tiny bias row goes after w0's first half: every DMA ahead
            # of w0a costs serial HWDGE+transfer time on the ramp. ----
            w0 = wst.tile([P, KK, GW], FP8, tag="w")
            nc.sync.dma_start(w0[:, :, 0:GW // 2], w8[:, :, 0:GW // 2])
            b0 = bst.tile([1, GW], FP8, tag="b")
            nc.sync.dma_start(b0[:], b8d[0:1, 0:GW])
            h8t = const.tile([P, KK, ROWS], FP8, tag="h8t")
            nc.sync.dma_start(h8t[:], h8[:, :, :])
            nc.sync.dma_start(w0[:, :, GW // 2:GW], w8[:, :, GW // 2:GW])
            ones16 = const.tile([1, P], FP16, tag="ones16")
            nc.vector.memset(ones16[:], 1.0)
            bcneg = const.tile([P, 1], FP32, tag="bcneg")
            nc.vector.memset(bcneg[:], -bc_value)
            accs = [
                const.tile([P, NG + 2], FP32, tag=f"acc{j}", name=f"acc{j}")
                for j in range(BPC)
            ]
            # Spare columns NG+g hold the half-B partials of split groups
            # g in (0,1); zero them everywhere first (accum_out overwrites).
            for j in range(BPC):
                nc.vector.memset(accs[j][:, NG:NG + 2], 0.0)

            exp_tiles = [[None] * NG for _ in range(BPC)]
            pcs = [None] * BPC
            ompc4 = [None] * BPC
            scales = [None] * BPC

            def emit_pc(pcps):
                """p_copy for all chunks into a group tile's slack columns
                (fp16 path, exp-based sigmoid to stay in the exp table set).
                Must be emitted BEFORE the host group's own matmuls so the
                slack isn't re-marked pending-zero afterwards."""
                for j in range(BPC):
                    js = slice(j * P, (j + 1) * P)
                    for kk in range(KK):
                        nc.tensor.matmul(
                            pcps[:, j:j + 1, 500:501],
                            h16t[:, kk:kk + 1, js],
                            wc16t[:, kk:kk + 1],
                            start=(kk == 0), stop=(kk == KK - 1),
                        )
                for j in range(BPC):
                    en = const.tile([P, 1], FP32, tag=f"en{j}", name=f"en{j}")
                    nc.scalar.activation(
                        en[:], pcps[:, j:j + 1, 500:501], AF.Exp,
                        bias=bcneg[:], scale=-1.0,
                    )
                    onep = const.tile([P, 1], FP32, tag=f"onep{j}", name=f"onep{j}")
                    nc.vector.tensor_scalar_add(onep[:], en[:], 1.0)
                    pc = const.tile([P, 1], FP32, tag=f"pc{j}", name=f"pc{j}")
                    nc.vector.reciprocal(pc[:], onep[:])          # sigmoid
                    om4 = const.tile([P, 1], FP32, tag=f"om4{j}", name=f"om4{j}")
                    nc.vector.tensor_mul(om4[:], en[:], pc[:])    # 1 - sigmoid
                    nc.vector.tensor_scalar_mul(om4[:], om4[:], SCALE_OUT)
                    pcs[j] = pc
                    ompc4[j] = om4

            def emit_copy(j, scale_eng):
                """copy/scatter part: p_copy * (attn_j @ src_map_j) -> fp16.
                scale_eng 0 = DVE, 2 = ACT (both can read psum)."""
                js = slice(j * P, (j + 1) * P)
                at, smt = copy_ins[j]
                cp = psp.tile([P, GPC, 512], FP32, tag="ps")
                for ks in range(KS):
                    nc.tensor.matmul(
                        cp[:, 0:1, 0:512], at[:, ks:ks + 1, :],
                        smt[:, ks:ks + 1, 0:512],
                        start=(ks == 0), stop=(ks == KS - 1),
                    )
                    nc.tensor.matmul(
                        cp[:, 1:2, 0:C - 512], at[:, ks:ks + 1, :],
                        smt[:, ks:ks + 1, 512:C],
                        start=(ks == 0), stop=(ks == KS - 1),
                    )
                cst = cpool.tile([P, C], FP16, tag="cst")
                if scale_eng == 0:
                    nc.vector.tensor_scalar_mul(cst[:, 0:512], cp[:, 0:1, 0:512], pcs[j][:])
                    nc.vector.tensor_scalar_mul(cst[:, 512:C], cp[:, 1:2, 0:C - 512], pcs[j][:])
                else:
                    nc.scalar.activation(cst[:, 0:512], cp[:, 0:1, 0:512], AF.Copy, scale=pcs[j][:])
                    nc.scalar.activation(cst[:, 512:C], cp[:, 1:2, 0:C - 512], AF.Copy, scale=pcs[j][:])
                nc.sync.dma_start(outC[js, :], cst[:])
                return js, cst

            copy_ins = [None] * BPC

            def emit_copy_loads(j):
                at = apool.tile([P, KS, T], FP16, tag="at")
                nc.sync.dma_start(at[:], a16[j, :, :, :])
                smt = smpool.tile([P, KS, C], FP8, tag="smt")
                nc.sync.dma_start(smt[:], sm8[j, :, :, :])
                copy_ins[j] = (at, smt)

            RES_GS = ()
            resident = {}

            def emit_wload(pas_g):
                """Queue the DMA for one W group (+ bias slice). Groups in
                RES_GS are loaded once in pass A into persistent tiles and
                reused by pass B, trimming pass-B DMA and removing the
                W-arrival wait at the end of the kernel."""
                pas, g = pas_g
                if pas == 1 and g in RES_GS:
                    return resident[g]
                gs = slice(g * GW, (g + 1) * GW)
                if pas == 0 and g in RES_GS:
                    wt = const.tile([P, KK, GW], FP8, tag=f"wres{g}", name=f"wres{g}")
                    bt = const.tile([1, GW], FP8, tag=f"bres{g}", name=f"bres{g}")
                    resident[g] = (wt, bt)
                else:
                    wt = wst.tile([P, KK, GW], FP8, tag="w")
                    bt = bst.tile([1, GW], FP8, tag="b")
                if pas == 0 and g == 1:
                    nc.sync.dma_start(wt[:, :, 0:GW // 2], w8[:, :, gs][:, :, 0:GW // 2])
                    nc.sync.dma_start(bt[:], b8d[0:1, gs])
                    nc.sync.dma_start(wt[:, :, GW // 2:GW], w8[:, :, gs][:, :, GW // 2:GW])
                else:
                    nc.sync.dma_start(wt[:], w8[:, :, gs])
                    nc.sync.dma_start(bt[:], b8d[0:1, gs])
                return wt, bt

            def emit_chunk(j, g, wt, bt, with_pc=False, split=False):
                """Matmul+exp for one chunk against a loaded W group.
                split: drain in two 2-bank halves so the very first exp
                starts as soon as the first half-W transfer lands."""
                js = slice(j * P, (j + 1) * P)
                ps = psp.tile([P, GPC, 512], FP32, tag="ps")
                if with_pc:
                    emit_pc(ps)
                halves = 2 if split else 1
                qph = GPC // halves
                et = expp.tile([P, GPC, NT], FP8, tag="e")
                for h in range(halves):
                    for q in range(h * qph, (h + 1) * qph):
                        cs = slice(q * NT, (q + 1) * NT)
                        nc.tensor.matmul(
                            ps[:, q:q + 1, 0:NT], ones16[0:1, :], bt[0:1, cs],
                            start=True, stop=False,
                        )
                        for kh in range(KK // 2):
                            nc.tensor.matmul(
                                ps[:, q:q + 1, 0:NT],
                                h8t[:, 2 * kh:2 * kh + 2, js],
                                wt[:, 2 * kh:2 * kh + 2, cs],
                                start=False, stop=(kh == KK // 2 - 1),
                                perf_mode=DR,
                            )
                    hq = slice(h * qph, (h + 1) * qph)
                    nc.scalar.activation(
                        et[:, hq, :], ps[:, hq, 0:NT], AF.Exp,
                        scale=1.0 / SCALE_W,
                    )
                    # Row-sum partial on DVE (x1.0 in place + accum),
                    # keeping ACT free of the ~187ns accum-read per tile.
                    # Split halves use the spare accum column NG.
                    acol = g if h == 0 else NG + g
                    nc.vector.tensor_scalar(
                        et[:, hq, :], et[:, hq, :], 1.0, None,
                        mybir.AluOpType.mult, mybir.AluOpType.add,
                        accum_out=accs[j][:, acol:acol + 1],
                    )
                exp_tiles[j][g] = et

            def emit_head(j):
                """Per-row output scale: 4096 * (1-p_copy) / rowsum."""
                rs = const.tile([P, 1], FP32, tag=f"rs{j}")
                nc.vector.reduce_sum(rs[:], accs[j][:], axis=mybir.AxisListType.X)
                rec = const.tile([P, 1], FP32, tag=f"rec{j}")
                nc.vector.reciprocal(rec[:], rs[:])
                s = const.tile([P, 1], FP32, tag=f"s{j}")
                nc.vector.tensor_mul(s[:], rec[:], ompc4[j][:])
                scales[j] = s

            def emit_scale(j, g, eng):
                """Scale one exp tile in place (0=DVE, 1=GPSIMD, 2=ACT)."""
                et = exp_tiles[j][g]
                if eng == 0:
                    nc.vector.tensor_scalar_mul(et[:], et[:], scales[j][:])
                elif eng == 1:
                    nc.gpsimd.tensor_scalar_mul(et[:], et[:], scales[j][:])
                else:
                    nc.scalar.activation(et[:], et[:], AF.Copy, scale=scales[j][:])

            def emit_store(j, g):
                """Store one scaled exp tile. Emitted a couple of ops after
                its scale: a dma_start whose source isn't ready blocks the
                SP sequencer FIFO and with it every later DMA issue."""
                js = slice(j * P, (j + 1) * P)
                nc.sync.dma_start(outV[js, g * GW:(g + 1) * GW], exp_tiles[j][g][:])

            # ---- main passes with W prefetch depth 2 across the pass
            # boundary (wst/bst pools are sized so loads run ~2 groups
            # ahead and pass B's first groups are in flight before pass A
            # drains). p_copy rides in group (A,2)'s psum slack; the
            # copy/scatter parts run in the tail where PE+psum are free. ----
            steps = [(pas, g) for pas in range(PASSES) for g in range(NG)]
            PF = 3
            loads = {0: (w0, b0)}
            lstate = {"next": 1}

            def issue_loads(upto):
                while lstate["next"] < min(upto, len(steps)):
                    loads[lstate["next"]] = emit_wload(steps[lstate["next"]])
                    lstate["next"] += 1

            issue_loads(PF)

            # Remaining resident constants (after the first W loads so the
            # ramp-critical transfers go out first).
            h16t = const.tile([P, KK, ROWS], FP16, tag="h16t")
            nc.sync.dma_start(h16t[:], h16[:, :, :])
            wc16t = const.tile([P, KK], FP16, tag="wc16t")
            nc.sync.dma_start(wc16t[:], wc16d[:, :])

            # Drain split for pass B: DVE/GPSIMD only (ACT is exp-bound).
            # DVE also accrues ~2.2us of accum work per slot, modeled by
            # bumping its busy counter each step so the greedy interleaves
            # instead of front-loading GPSIMD.
            drain = [(cj, g) for g in range(NG) for cj in range(CPP)]
            dr_engs = []
            busy = [0.0, 0.0]
            cost2 = [1102.0, 2968.0]
            for s in range(NG):
                busy[0] += 2204.0
                for _ in range(CPP):
                    eng = min(range(2), key=lambda e: busy[e] + cost2[e])
                    dr_engs.append(eng)
                    busy[eng] += cost2[eng]
            di = 0
            store_q = []          # (j, g) scaled but not yet stored
            STORE_LAG = 7

            # ---- pass A: chunks 0,1 group-by-group ----
            for g in range(NG):
                wt, bt = loads.pop(g)
                issue_loads(g + 1 + PF)
                emit_chunk(0, g, wt, bt, with_pc=(g == 2), split=(g <= 1))
                emit_chunk(1, g, wt, bt, split=(g <= 1))
                if g in (5, 7, 9, 11):
                    emit_copy_loads((g - 5) // 2)
            for j in range(CPP):
                emit_head(j)

            # ---- pass B: chunk 3 staggered K_STAG groups behind chunk 2,
            # so chunk 2's row-sum completes early and its scale+stores
            # overlap chunk 3's last exps. W tiles stay alive one rotation
            # longer (wst/bst bufs cover PF-ahead + K_STAG-behind). ----
            K_STAG = 2
            for s in range(NG + K_STAG):
                issue_loads(NG + s + K_STAG + 1)
                if s < NG:
                    wt, bt = loads[NG + s]
                    emit_chunk(2, s, wt, bt)
                if s >= K_STAG:
                    gg = s - K_STAG
                    wt, bt = loads.pop(NG + gg)
                    emit_chunk(3, gg, wt, bt)
                if s < NG:
                    for _ in range(CPP):
                        j, gg2 = drain[di]
                        emit_scale(j, gg2, dr_engs[di])
                        store_q.append((j, gg2))
                        di += 1
                    while len(store_q) > STORE_LAG:
                        emit_store(*store_q.pop(0))
                if s == NG - 1:
                    emit_head(CPP)
            emit_head(CPP + 1)

            # ---- tail: copy parts (PE+psum now free; scale DVE/ACT) and
            # chunks 2,3 scale+store across DVE/GPSIMD/ACT, greedy-balanced
            # (per-tile ns: DVE 1102, GPSIMD 2968, ACT 1852) ----
            while store_q:
                emit_store(*store_q.pop(0))
            copy_sts = [emit_copy(j, scale_eng=0 if j < 2 else 2)
                        for j in range(BPC)]
            tail_engs = []
            busy3 = [3.1e3, 0.0, 3.4e3]   # seeded with the copy-scale work
            cost3 = [1102.0, 2968.0, 1852.0]
            for _ in range(2 * NG):
                eng = min(range(3), key=lambda e: busy3[e] + cost3[e])
                tail_engs.append(eng)
                busy3[eng] += cost3[eng]
            # No W loads remain, so a store blocking the SP queue can only
            # delay other stores - emit each store right after its scale;
            # the copy parts' outC stores go out once their data is ready.
            ti = 0
            for cj in range(CPP):
                for g in range(NG):
                    emit_scale(CPP + cj, g, tail_engs[ti])
                    emit_store(CPP + cj, g)
                    ti += 1


    nc.finalize()
    return nc


def _prep_inputs(hidden, attn, W, b, Wc, bc, src_map):
    """Host-side shard + layout prep. Returns per-core input maps and bc."""
    hidden, attn, W, b, Wc, bc, src_map = (
        np.asarray(x) for x in (hidden, attn, W, b, Wc, bc, src_map)
    )
    E4 = ml_dtypes.float8_e4m3
    f16 = np.float16

    # W^T * 64 in [p, kk, v] layout, fp8e4, replicated to all cores
    wT = (W.T.astype(np.float32) * SCALE_W).reshape(KK, P, V)
    w8 = np.ascontiguousarray(np.clip(wT, -240, 240).transpose(1, 0, 2)).astype(E4)
    b8 = np.clip(b.astype(np.float32) * SCALE_W, -240, 240).astype(E4).reshape(1, V)
    wc16 = np.ascontiguousarray(Wc[0].reshape(KK, P).T).astype(f16)  # (P, KK)

    hid = hidden.reshape(T, B, H)     # [t, b, h]
    att = attn.reshape(T, B, S)       # [t, b, s]

    in_maps = []
    for k in range(NCORES):
        bs = slice(k * BPC, (k + 1) * BPC)
        # local rows r = j*128 + t (j = local batch idx, t = time)
        hk = hid[:, bs, :].transpose(1, 0, 2).reshape(ROWS, H)   # [r, h]
        hT = hk.T.reshape(KK, P, ROWS).transpose(1, 0, 2)        # [p, kk, r]
        h8_k = np.ascontiguousarray(np.clip(hT, -240, 240)).astype(E4)
        h16_k = np.ascontiguousarray(hT).astype(f16)

        aT = att[:, bs, :].transpose(1, 2, 0)                    # (BPC, S, T)
        aP = np.zeros((BPC, SPAD, T), np.float32)
        aP[:, :S] = aT
        a16_k = np.ascontiguousarray(
            aP.reshape(BPC, KS, P, T).transpose(0, 2, 1, 3)).astype(f16)

        sK = src_map[:, bs, :].transpose(1, 0, 2)                # (BPC, S, C)
        sP = np.zeros((BPC, SPAD, C), np.float32)
        sP[:, :S] = sK
        sm8_k = np.ascontiguousarray(
            sP.reshape(BPC, KS, P, C).transpose(0, 2, 1, 3)).astype(E4)

        in_maps.append({"h8": h8_k, "h16": h16_k, "w8": w8, "b8": b8,
                        "wc16": wc16, "a16": a16_k, "sm8": sm8_k})
    return in_maps, float(bc[0])


def _assemble(results):
    """Per-core fp8 V-part (descale by 4096) + fp16 copy part -> (4096, 32620)."""
    Vp = np.stack([np.asarray(r["outV"]) for r in results])      # (8, 512, V)
    Cp = np.stack([np.asarray(r["outC"]) for r in results])      # (8, 512, C)
    out = np.empty((NCORES, ROWS, V + C), np.float32)
    out[:, :, :V] = Vp.astype(np.float32) * (1.0 / SCALE_OUT)
    out[:, :, V:] = Cp.astype(np.float32)
    out = out.reshape(NCORES, BPC, T, V + C)
    out = out.transpose(2, 0, 1, 3).reshape(N, V + C)            # row = t*32 + (4k+j)
    return np.ascontiguousarray(out)


_CACHE = {}


def _run(inputs, **spmd_kwargs):
    in_maps, bc_value = _prep_inputs(**inputs)
    key = round(bc_value, 12)
    if key not in _CACHE:
        _CACHE[key] = build_kernel(bc_value)
    nc = _CACHE[key]
    res = run_bass_kernel_spmd(
        nc, in_maps, core_ids=list(range(NCORES)), **spmd_kwargs
    )
    return _assemble(res.results), res


def kernel(**inputs):
    out, _ = _run(inputs)
    return out


# revision 64
# speedup vs baseline: 2.6369x; 1.0005x over previous
"""CopyGenerator kernel for Trainium2, SPMD over 8 NeuronCores.

Problem (nn_CopyGenerator):
    logits = hidden @ W.T + b            # (N=4096, V=32000)
    prob   = softmax(logits, axis=1)
    p_copy = sigmoid(hidden @ Wc.T + bc) # (N, 1)
    out    = [prob * (1 - p_copy),  scatter(attn * p_copy)]   # (N, 32620)

Sharding: data-parallel over the batch axis. Core k handles batch elements
{4k..4k+3} (4 row-chunks of 128, local row r = j*128 + t).

Strategy (memory-regime; all numbers per core):
  * W is pre-scaled by 64 and stored as fp8e4 (16.4 MB instead of 32.8 fp16),
    hidden likewise; the vocab GEMM runs in DoubleRow fp8 (two 128-deep
    k-subtiles per matmul) which halves PE cycles again. The x64 scale is
    undone by the Exp activation's input scale.
  * The bias row (x64, fp8) is applied per 500-col tile via a K=1 matmul of
    fp16 ones against the streamed bias row.
  * exp(logits) goes to fp8 SBUF tiles; row-sum partials come from a DVE
    in-place x1.0 with accum_out (keeping ACT free of accum reads). V-part
    output is stored as fp8 scaled by 4096 (the host divides it back out),
    the 620-col copy part as fp16 - 4x less store traffic than fp32.
  * W streams twice (pass A: chunks 0,1; pass B: chunks 2,3). Chunks 0,1 are
    scaled (DVE+GPSIMD) and stored while pass B computes; in pass B chunk 3
    runs 2 groups behind chunk 2 (W tiles live one rotation longer), so
    chunk 2's row-sum closes early and its stores overlap chunk 3's last
    exps; the tail scales the rest on DVE+GPSIMD+ACT. Stores are emitted a
    few ops behind their scales: a dma_start with an unready source blocks
    the SP issue FIFO and every later DMA behind it.
  * W groups 0-1 load in half-width transfers and drain in two 2-bank ACT
    halves (spare accum columns NG+g), pulling the exp stream start to
    ~5us. The W pool holds 6 buffers so ~5 pass-B loads are in flight
    across the pass boundary (filling what was a ~10us DMA hole there);
    after the W stream ends the kernel is ~94% store-bound.
  * p_copy uses a separate fp16 hidden copy for accuracy; sigmoid is computed
    as 1/(1+exp(-x)) on ACT+DVE to stay in the exp table set.
"""

import numpy as np
import ml_dtypes

import concourse.bass as bass
import concourse.mybir as mybir
import concourse.tile as tile
from concourse import bacc
from concourse.bass_utils import run_bass_kernel_spmd

# Problem constants (hardcoded per contract)
B, T, S, H, V, C = 32, 128, 400, 512, 32000, 620
N = B * T
NCORES = 8
BPC = B // NCORES          # batch elems (row-chunks) per core = 4
ROWS = BPC * T             # rows per core = 512
P = 128                    # partitions
KK = H // P                # 4 contraction subtiles of 128
NT = 500                   # psum n-tile width (bank holds 512 fp32)
GPC = 4                    # n-tiles per psum group
GW = NT * GPC              # 2000 cols per group
NG = V // GW               # 16 groups per chunk
SPAD = 512                 # source len padded to 4 subtiles
KS = SPAD // P             # 4
PASSES = 2
CPP = BPC // PASSES        # chunks per pass = 2

SCALE_W = 64.0             # W/b pre-scale (better fp8e4 range)
SCALE_OUT = 4096.0         # V-part output scale (host divides back)

FP8 = mybir.dt.float8e4
FP16 = mybir.dt.float16
FP32 = mybir.dt.float32
AF = mybir.ActivationFunctionType
DR = mybir.MatmulPerfMode.DoubleRow


def build_kernel(bc_value: float):
    nc = bacc.Bacc("TRN2", target_bir_lowering=False)

    h8 = nc.dram_tensor("h8", (P, KK, ROWS), FP8, kind="ExternalInput")
    h16 = nc.dram_tensor("h16", (P, KK, ROWS), FP16, kind="ExternalInput")
    w8 = nc.dram_tensor("w8", (P, KK, V), FP8, kind="ExternalInput")
    b8d = nc.dram_tensor("b8", (1, V), FP8, kind="ExternalInput")
    wc16d = nc.dram_tensor("wc16", (P, KK), FP16, kind="ExternalInput")
    a16 = nc.dram_tensor("a16", (BPC, P, KS, T), FP16, kind="ExternalInput")
    sm8 = nc.dram_tensor("sm8", (BPC, P, KS, C), FP8, kind="ExternalInput")
    outV = nc.dram_tensor("outV", (ROWS, V), FP8, kind="ExternalOutput")
    outC = nc.dram_tensor("outC", (ROWS, C), FP16, kind="ExternalOutput")

    with tile.TileContext(nc) as tc:
        with (
            tc.tile_pool(name="const", bufs=1) as const,
            tc.tile_pool(name="wst", bufs=6) as wst,
            tc.tile_pool(name="bst", bufs=6) as bst,
            tc.tile_pool(name="expp", bufs=PASSES * CPP * NG) as expp,
            tc.tile_pool(name="apool", bufs=4) as apool,
            tc.tile_pool(name="smpool", bufs=4) as smpool,
            tc.tile_pool(name="cpool", bufs=1) as cpool,
            tc.tile_pool(name="psp", bufs=2, space="PSUM") as psp,
        ):
            # ---- ramp-critical loads first: W group 0 in two half-width
            # transfers (the first exp only needs the first half), then h8.
            # The tiny bias row goes after w0's first half: every DMA ahead
            # of w0a costs serial HWDGE+transfer time on the ramp. ----
            w0 = wst.tile([P, KK, GW], FP8, tag="w")
            nc.sync.dma_start(w0[:, :, 0:GW // 2], w8[:, :, 0:GW // 2])
            b0 = bst.tile([1, GW], FP8, tag="b")
            nc.sync.dma_start(b0[:], b8d[0:1, 0:GW])
            h8t = const.tile([P, KK, ROWS], FP8, tag="h8t")
            nc.sync.dma_start(h8t[:], h8[:, :, :])
            nc.sync.dma_start(w0[:, :, GW // 2:GW], w8[:, :, GW // 2:GW])
            ones16 = const.tile([1, P], FP16, tag="ones16")
            nc.vector.memset(ones16[:], 1.0)
            bcneg = const.tile([P, 1], FP32, tag="bcneg")
            nc.vector.memset(bcneg[:], -bc_value)
            accs = [
                const.tile([P, NG + 2], FP32, tag=f"acc{j}", name=f"acc{j}")
                for j in range(BPC)
            ]
            # Spare columns NG+g hold the half-B partials of split groups
            # g in (0,1); zero them everywhere first (accum_out overwrites).
            for j in range(BPC):
                nc.vector.memset(accs[j][:, NG:NG + 2], 0.0)

            exp_tiles = [[None] * NG for _ in range(BPC)]
            pcs = [None] * BPC
            ompc4 = [None] * BPC
            scales = [None] * BPC

            def emit_pc(pcps):
                """p_copy for all chunks into a group tile's slack columns
                (fp16 path, exp-based sigmoid to stay in the exp table set).
                Must be emitted BEFORE the host group's own matmuls so the
                slack isn't re-marked pending-zero afterwards."""
                for j in range(BPC):
                    js = slice(j * P, (j + 1) * P)
                    for kk in range(KK):
                        nc.tensor.matmul(
                            pcps[:, j:j + 1, 500:501],
                            h16t[:, kk:kk + 1, js],
                            wc16t[:, kk:kk + 1],
                            start=(kk == 0), stop=(kk == KK - 1),
                        )
                for j in range(BPC):
                    en = const.tile([P, 1], FP32, tag=f"en{j}", name=f"en{j}")
                    nc.scalar.activation(
                        en[:], pcps[:, j:j + 1, 500:501], AF.Exp,
                        bias=bcneg[:], scale=-1.0,
                    )
                    onep = const.tile([P, 1], FP32, tag=f"onep{j}", name=f"onep{j}")
                    nc.vector.tensor_scalar_add(onep[:], en[:], 1.0)
                    pc = const.tile([P, 1], FP32, tag=f"pc{j}", name=f"pc{j}")
                    nc.vector.reciprocal(pc[:], onep[:])          # sigmoid
                    om4 = const.tile([P, 1], FP32, tag=f"om4{j}", name=f"om4{j}")
                    nc.vector.tensor_mul(om4[:], en[:], pc[:])    # 1 - sigmoid
                    nc.vector.tensor_scalar_mul(om4[:], om4[:], SCALE_OUT)
                    pcs[j] = pc
                    ompc4[j] = om4

            def emit_copy(j, scale_eng):
                """copy/scatter part: p_copy * (attn_j @ src_map_j) -> fp16.
                scale_eng 0 = DVE, 2 = ACT (both can read psum)."""
                js = slice(j * P, (j + 1) * P)
                at, smt = copy_ins[j]
                cp = psp.tile([P, GPC, 512], FP32, tag="ps")
                for ks in range(KS):
                    nc.tensor.matmul(
                        cp[:, 0:1, 0:512], at[:, ks:ks + 1, :],
                        smt[:, ks:ks + 1, 0:512],
                        start=(ks == 0), stop=(ks == KS - 1),
                    )
                    nc.tensor.matmul(
                        cp[:, 1:2, 0:C - 512], at[:, ks:ks + 1, :],
                        smt[:, ks:ks + 1, 512:C],
                        start=(ks == 0), stop=(ks == KS - 1),
                    )
                cst = cpool.tile([P, C], FP16, tag="cst")
                if scale_eng == 0:
                    nc.vector.tensor_scalar_mul(cst[:, 0:512], cp[:, 0:1, 0:512], pcs[j][:])
                    nc.vector.tensor_scalar_mul(cst[:, 512:C], cp[:, 1:2, 0:C - 512], pcs[j][:])
                else:
                    nc.scalar.activation(cst[:, 0:512], cp[:, 0:1, 0:512], AF.Copy, scale=pcs[j][:])
                    nc.scalar.activation(cst[:, 512:C], cp[:, 1:2, 0:C - 512], AF.Copy, scale=pcs[j][:])
                return js, cst

            copy_ins = [None] * BPC

            def emit_copy_loads(j):
                at = apool.tile([P, KS, T], FP16, tag="at")
                nc.sync.dma_start(at[:], a16[j, :, :, :])
                smt = smpool.tile([P, KS, C], FP8, tag="smt")
                nc.sync.dma_start(smt[:], sm8[j, :, :, :])
                copy_ins[j] = (at, smt)

            RES_GS = ()
            resident = {}

            def emit_wload(pas_g):
                """Queue the DMA for one W group (+ bias slice). Groups in
                RES_GS are loaded once in pass A into persistent tiles and
                reused by pass B, trimming pass-B DMA and removing the
                W-arrival wait at the end of the kernel."""
                pas, g = pas_g
                if pas == 1 and g in RES_GS:
                    return resident[g]
                gs = slice(g * GW, (g + 1) * GW)
                if pas == 0 and g in RES_GS:
                    wt = const.tile([P, KK, GW], FP8, tag=f"wres{g}", name=f"wres{g}")
                    bt = const.tile([1, GW], FP8, tag=f"bres{g}", name=f"bres{g}")
                    resident[g] = (wt, bt)
                else:
                    wt = wst.tile([P, KK, GW], FP8, tag="w")
                    bt = bst.tile([1, GW], FP8, tag="b")
                if pas == 0 and g == 1:
                    nc.sync.dma_start(wt[:, :, 0:GW // 2], w8[:, :, gs][:, :, 0:GW // 2])
                    nc.sync.dma_start(bt[:], b8d[0:1, gs])
                    nc.sync.dma_start(wt[:, :, GW // 2:GW], w8[:, :, gs][:, :, GW // 2:GW])
                else:
                    nc.sync.dma_start(wt[:], w8[:, :, gs])
                    nc.sync.dma_start(bt[:], b8d[0:1, gs])
                return wt, bt

            def emit_chunk(j, g, wt, bt, with_pc=False, split=False):
                """Matmul+exp for one chunk against a loaded W group.
                split: drain in two 2-bank halves so the very first exp
                starts as soon as the first half-W transfer lands."""
                js = slice(j * P, (j + 1) * P)
                ps = psp.tile([P, GPC, 512], FP32, tag="ps")
                if with_pc:
                    emit_pc(ps)
                halves = 2 if split else 1
                qph = GPC // halves
                et = expp.tile([P, GPC, NT], FP8, tag="e")
                for h in range(halves):
                    for q in range(h * qph, (h + 1) * qph):
                        cs = slice(q * NT, (q + 1) * NT)
                        nc.tensor.matmul(
                            ps[:, q:q + 1, 0:NT], ones16[0:1, :], bt[0:1, cs],
                            start=True, stop=False,
                        )
                        for kh in range(KK // 2):
                            nc.tensor.matmul(
                                ps[:, q:q + 1, 0:NT],
                                h8t[:, 2 * kh:2 * kh + 2, js],
                                wt[:, 2 * kh:2 * kh + 2, cs],
                                start=False, stop=(kh == KK // 2 - 1),
                                perf_mode=DR,
                            )
                    hq = slice(h * qph, (h + 1) * qph)
                    nc.scalar.activation(
                        et[:, hq, :], ps[:, hq, 0:NT], AF.Exp,
                        scale=1.0 / SCALE_W,
                    )
                    # Row-sum partial on DVE (x1.0 in place + accum),
                    # keeping ACT free of the ~187ns accum-read per tile.
                    # Split halves use the spare accum column NG.
                    acol = g if h == 0 else NG + g
                    nc.vector.tensor_scalar(
                        et[:, hq, :], et[:, hq, :], 1.0, None,
                        mybir.AluOpType.mult, mybir.AluOpType.add,
                        accum_out=accs[j][:, acol:acol + 1],
                    )
                exp_tiles[j][g] = et

            def emit_head(j):
                """Per-row output scale: 4096 * (1-p_copy) / rowsum."""
                rs = const.tile([P, 1], FP32, tag=f"rs{j}")
                nc.vector.reduce_sum(rs[:], accs[j][:], axis=mybir.AxisListType.X)
                rec = const.tile([P, 1], FP32, tag=f"rec{j}")
                nc.vector.reciprocal(rec[:], rs[:])
                s = const.tile([P, 1], FP32, tag=f"s{j}")
                nc.vector.tensor_mul(s[:], rec[:], ompc4[j][:])
                scales[j] = s

            def emit_scale(j, g, eng):
                """Scale one exp tile in place (0=DVE, 1=GPSIMD, 2=ACT)."""
                et = exp_tiles[j][g]
                if eng == 0:
                    nc.vector.tensor_scalar_mul(et[:], et[:], scales[j][:])
                elif eng == 1:
                    nc.gpsimd.tensor_scalar_mul(et[:], et[:], scales[j][:])
                else:
                    nc.scalar.activation(et[:], et[:], AF.Copy, scale=scales[j][:])

            def emit_store(j, g):
                """Store one scaled exp tile. Emitted a couple of ops after
                its scale: a dma_start whose source isn't ready blocks the
                SP sequencer FIFO and with it every later DMA issue."""
                js = slice(j * P, (j + 1) * P)
                nc.sync.dma_start(outV[js, g * GW:(g + 1) * GW], exp_tiles[j][g][:])

            # ---- main passes with W prefetch depth 2 across the pass
            # boundary (wst/bst pools are sized so loads run ~2 groups
            # ahead and pass B's first groups are in flight before pass A
            # drains). p_copy rides in group (A,2)'s psum slack; the
            # copy/scatter parts run in the tail where PE+psum are free. ----
            steps = [(pas, g) for pas in range(PASSES) for g in range(NG)]
            PF = 3
            loads = {0: (w0, b0)}
            lstate = {"next": 1}

            def issue_loads(upto):
                while lstate["next"] < min(upto, len(steps)):
                    loads[lstate["next"]] = emit_wload(steps[lstate["next"]])
                    lstate["next"] += 1

            issue_loads(PF)

            # Remaining resident constants (after the first W loads so the
            # ramp-critical transfers go out first).
            h16t = const.tile([P, KK, ROWS], FP16, tag="h16t")
            nc.sync.dma_start(h16t[:], h16[:, :, :])
            wc16t = const.tile([P, KK], FP16, tag="wc16t")
            nc.sync.dma_start(wc16t[:], wc16d[:, :])

            # Drain split for pass B: DVE/GPSIMD only (ACT is exp-bound).
            # DVE also accrues ~2.2us of accum work per slot, modeled by
            # bumping its busy counter each step so the greedy interleaves
            # instead of front-loading GPSIMD.
            drain = [(cj, g) for g in range(NG) for cj in range(CPP)]
            dr_engs = []
            busy = [0.0, 0.0]
            cost2 = [1102.0, 2968.0]
            for s in range(NG):
                busy[0] += 2204.0
                for _ in range(CPP):
                    eng = min(range(2), key=lambda e: busy[e] + cost2[e])
                    dr_engs.append(eng)
                    busy[eng] += cost2[eng]
            di = 0
            store_q = []          # (j, g) scaled but not yet stored
            STORE_LAG = 7

            # ---- pass A: chunks 0,1 group-by-group ----
            for g in range(NG):
                wt, bt = loads.pop(g)
                # Near the pass boundary, burst the prefetch window: pass-A
                # buffers are freed as their groups drain, so up to 5 pass-B
                # W loads can be in flight across the transition, filling
                # what was a ~10us DMA hole there.
                issue_loads(g + 1 + PF if g < NG - 2 else NG + 5)
                emit_chunk(0, g, wt, bt, with_pc=(g == 2), split=(g <= 1))
                emit_chunk(1, g, wt, bt, split=(g <= 1))
                if g in (5, 7, 9, 11):
                    emit_copy_loads((g - 5) // 2)
            for j in range(CPP):
                emit_head(j)

            # ---- pass B: chunk 3 staggered K_STAG groups behind chunk 2,
            # so chunk 2's row-sum completes early and its scale+stores
            # overlap chunk 3's last exps. W tiles stay alive one rotation
            # longer (wst/bst bufs cover PF-ahead + K_STAG-behind). ----
            K_STAG = 2
            for s in range(NG + K_STAG):
                issue_loads(NG + s + K_STAG + 1)
                if s < NG:
                    wt, bt = loads[NG + s]
                    emit_chunk(2, s, wt, bt)
                if s >= K_STAG:
                    gg = s - K_STAG
                    wt, bt = loads.pop(NG + gg)
                    emit_chunk(3, gg, wt, bt)
                if s < NG:
                    for _ in range(CPP):
                        j, gg2 = drain[di]
                        emit_scale(j, gg2, dr_engs[di])
                        store_q.append((j, gg2))
                        di += 1
                    while len(store_q) > STORE_LAG:
                        emit_store(*store_q.pop(0))
                if s == NG - 1:
                    emit_head(CPP)
            emit_head(CPP + 1)

            # ---- tail: copy parts (PE+psum now free; scale DVE/ACT) and
            # chunks 2,3 scale+store across DVE/GPSIMD/ACT, greedy-balanced
            # (per-tile ns: DVE 1102, GPSIMD 2968, ACT 1852) ----
            while store_q:
                emit_store(*store_q.pop(0))
            copy_sts = [emit_copy(j, scale_eng=0 if j < 2 else 2)
                        for j in range(BPC)]

            # No W loads remain, so a store blocking the SP queue can only
            # delay other stores - emit each store right after its scale;
            # the copy parts' outC stores go out once their data is ready.
            # Chunk 2 first (its head closed early thanks to the stagger),
            # then the copy parts, then chunk 3 - so no not-yet-ready store
            # blocks ready ones in the SP FIFO.
            tail_engs = []
            busy3 = [3.1e3, 0.0, 3.4e3]   # seeded with the copy-scale work
            cost3 = [1102.0, 2968.0, 1852.0]
            for _ in range(2 * NG):
                eng = min(range(3), key=lambda e: busy3[e] + cost3[e])
                tail_engs.append(eng)
                busy3[eng] += cost3[eng]
            ti = 0
            for cj in range(CPP):
                for g in range(NG):
                    emit_scale(CPP + cj, g, tail_engs[ti])
                    emit_store(CPP + cj, g)
                    ti += 1
                    if cj == 0 and g == 7:
                        for js_, cst_ in copy_sts:
                            nc.sync.dma_start(outC[js_, :], cst_[:])


    nc.finalize()
    return nc


def _prep_inputs(hidden, attn, W, b, Wc, bc, src_map):
    """Host-side shard + layout prep. Returns per-core input maps and bc."""
    hidden, attn, W, b, Wc, bc, src_map = (
        np.asarray(x) for x in (hidden, attn, W, b, Wc, bc, src_map)
    )
    E4 = ml_dtypes.float8_e4m3
    f16 = np.float16

    # W^T * 64 in [p, kk, v] layout, fp8e4, replicated to all cores
    wT = (W.T.astype(np.float32) * SCALE_W).reshape(KK, P, V)
    w8 = np.ascontiguousarray(np.clip(wT, -240, 240).transpose(1, 0, 2)).astype(E4)
    b8 = np.clip(b.astype(np.float32) * SCALE_W, -240, 240).astype(E4).reshape(1, V)
    wc16 = np.ascontiguousarray(Wc[0].reshape(KK, P).T).astype(f16)  # (P, KK)

    hid = hidden.reshape(T, B, H)     # [t, b, h]
    att = attn.reshape(T, B, S)       # [t, b, s]

    in_maps = []
    for k in range(NCORES):
        bs = slice(k * BPC, (k + 1) * BPC)
        # local rows r = j*128 + t (j = local batch idx, t = time)
        hk = hid[:, bs, :].transpose(1, 0, 2).reshape(ROWS, H)   # [r, h]
        hT = hk.T.reshape(KK, P, ROWS).transpose(1, 0, 2)        # [p, kk, r]
        h8_k = np.ascontiguousarray(np.clip(hT, -240, 240)).astype(E4)
        h16_k = np.ascontiguousarray(hT).astype(f16)

        aT = att[:, bs, :].transpose(1, 2, 0)                    # (BPC, S, T)
        aP = np.zeros((BPC, SPAD, T), np.float32)
        aP[:, :S] = aT
        a16_k = np.ascontiguousarray(
            aP.reshape(BPC, KS, P, T).transpose(0, 2, 1, 3)).astype(f16)

        sK = src_map[:, bs, :].transpose(1, 0, 2)                # (BPC, S, C)
        sP = np.zeros((BPC, SPAD, C), np.float32)
        sP[:, :S] = sK
        sm8_k = np.ascontiguousarray(
            sP.reshape(BPC, KS, P, C).transpose(0, 2, 1, 3)).astype(E4)

        in_maps.append({"h8": h8_k, "h16": h16_k, "w8": w8, "b8": b8,
                        "wc16": wc16, "a16": a16_k, "sm8": sm8_k})
    return in_maps, float(bc[0])


def _assemble(results):
    """Per-core fp8 V-part (descale by 4096) + fp16 copy part -> (4096, 32620)."""
    Vp = np.stack([np.asarray(r["outV"]) for r in results])      # (8, 512, V)
    Cp = np.stack([np.asarray(r["outC"]) for r in results])      # (8, 512, C)
    out = np.empty((NCORES, ROWS, V + C), np.float32)
    out[:, :, :V] = Vp.astype(np.float32) * (1.0 / SCALE_OUT)
    out[:, :, V:] = Cp.astype(np.float32)
    out = out.reshape(NCORES, BPC, T, V + C)
    out = out.transpose(2, 0, 1, 3).reshape(N, V + C)            # row = t*32 + (4k+j)
    return np.ascontiguousarray(out)


_CACHE = {}


def _run(inputs, **spmd_kwargs):
    in_maps, bc_value = _prep_inputs(**inputs)
    key = round(bc_value, 12)
    if key not in _CACHE:
        _CACHE[key] = build_kernel(bc_value)
    nc = _CACHE[key]
    res = run_bass_kernel_spmd(
        nc, in_maps, core_ids=list(range(NCORES)), **spmd_kwargs
    )
    return _assemble(res.results), res


def kernel(**inputs):
    out, _ = _run(inputs)
    return out
